# revision 1
# baseline (speedup 1.0000x reference)
"""Recurrent linear-attention transformer on 8 Trainium2 NeuronCores.

Sharding: 4-way data parallel over batch x 2-way sequence split (halves of
S=2048). Causal linear attention is computed in chunked form; the only
cross-core dependency is the cumulative (k^T v, sum k) state at the half
boundary, exchanged once per layer via a pairwise AllReduce.

Per-core layout conventions (SBUF 2-D tensors, 128 partitions):
  lat   f32  [128, 8*512]    seq-major residual: c-tile ct8 block, cols = e
  hT    bf16 [128, 4*4*256]  feature-major LN1 output: (chunk, kt) blocks
  kT    bf16 [128, 4*4*256]  phi(k) feature-major: (chunk, ft) blocks, cols=t
  v     bf16 [128, 4*2*512]  v seq-major: (chunk, tt) blocks, cols = e'
  P_all bf16 [128, 5*4*513]  prefix states: (j, kt) -> [KV[e,e'] | s_k[e]]
All matmul operands bf16, PSUM/stats/residual f32.
"""

import threading

import numpy as np
import ml_dtypes

import concourse.bass as bass
import concourse.bacc as bacc
import concourse.tile as tile
import concourse.mybir as mybir
from concourse.bass_utils import run_bass_kernel_spmd

AF = mybir.ActivationFunctionType
ALU = mybir.AluOpType
F32 = mybir.dt.float32
BF16 = mybir.dt.bfloat16
BF = ml_dtypes.bfloat16

L, B, CIN, COUT, E, S = 4, 4, 64, 64, 512, 2048
NCORES = 8
SH = S // 2          # per-core sequence half
C = 256              # attention chunk
NCH = SH // C        # 4 chunks
CT = C // 128        # 2 c-tiles per chunk
KT = E // 128        # 4 feature tiles
NT8 = SH // 128      # 8 seq tiles per half
EPS = 1e-6
LN_EPS = 1e-5
SKW = E + 1          # 513: KV block plus s_k column

REPLICA_GROUPS = [[0, 1], [2, 3], [4, 5], [6, 7]]

# brow layout (bf16): per-layer [bv, bo, c2b] rows, then in_b, cc_b, out_b
BROW_N = L * 3 * 512 + 512 + 512 + COUT


def build_program(cc=True, repeat=1, with_bias=False):
    nc = bacc.Bacc("TRN2", target_bir_lowering=False, debug=False,
                   num_devices=NCORES)

    x_d = nc.dram_tensor("x_sl", [CIN, SH + 2], BF16, kind="ExternalInput")
    inWT_d = nc.dram_tensor("inWT", [CIN, E], BF16, kind="ExternalInput")
    ccWT_d = nc.dram_tensor("ccWT", [128, 3 * KT * 512], BF16, kind="ExternalInput")
    wpack_d = nc.dram_tensor("wpack", [128, L * 6 * KT * 512], BF16, kind="ExternalInput")
    outWT_d = nc.dram_tensor("outWT", [128, KT * COUT], BF16, kind="ExternalInput")
    brow_d = nc.dram_tensor("brow", [1, BROW_N], BF16, kind="ExternalInput")
    bcol_d = nc.dram_tensor("bcol", [128, L * 3 * KT], F32, kind="ExternalInput")
    tril_d = nc.dram_tensor("tril", [128, 128], BF16, kind="ExternalInput")
    mcol_d = nc.dram_tensor("mcol", [128, 2], F32, kind="ExternalInput")  # [m, 1-m]
    halo_d = nc.dram_tensor("halo", [1, SH + 2], BF16, kind="ExternalInput")
    ones_row_d = nc.dram_tensor("ones_row", [1, 512], BF16, kind="ExternalInput")
    ones_col_bf_d = nc.dram_tensor("ones_col_bf", [128, 1], BF16, kind="ExternalInput")

    out_d = nc.dram_tensor("out", [COUT, SH], F32, kind="ExternalOutput")

    with tile.TileContext(nc, num_cores=NCORES) as tc:
        _emit(nc, tc, x_d, inWT_d, ccWT_d, wpack_d, outWT_d, brow_d, bcol_d,
              tril_d, mcol_d, halo_d, ones_row_d,
              ones_col_bf_d, out_d, cc=cc, repeat=repeat,
              with_bias=with_bias)
    nc.compile()
    return nc


def _emit(nc, tc, x_d, inWT_d, ccWT_d, wpack_d, outWT_d, brow_d, bcol_d,
          tril_d, mcol_d, halo_d, ones_row_d,
          ones_col_bf_d, out_d, cc=True, repeat=1,
          with_bias=False):
    import contextlib
    ctx = contextlib.ExitStack()
    with ctx:
        singles = ctx.enter_context(tc.tile_pool(name="singles", bufs=1))
        persist = ctx.enter_context(tc.tile_pool(name="persist", bufs=1))
        small = ctx.enter_context(tc.tile_pool(name="small", bufs=4))
        psum = ctx.enter_context(tc.tile_pool(name="psum", bufs=2, space="PSUM"))
        dram = ctx.enter_context(tc.tile_pool(name="dram", bufs=2, space="DRAM"))

        dma = nc.sync.dma_start

        # ---- constants ----
        outWT = singles.tile([128, KT * COUT], BF16)
        dma(out=outWT, in_=outWT_d[:, :])
        brow = singles.tile([1, BROW_N], BF16)
        dma(out=brow, in_=brow_d[:, :])
        bcol = singles.tile([128, L * 3 * KT], F32)
        dma(out=bcol, in_=bcol_d[:, :])
        tril = singles.tile([128, 128], BF16)
        dma(out=tril, in_=tril_d[:, :])
        mcol = singles.tile([128, 2], F32)
        dma(out=mcol, in_=mcol_d[:, :])
        halo = singles.tile([1, SH + 2], BF16)
        dma(out=halo, in_=halo_d[:, :])
        ones_row = singles.tile([1, 512], BF16)
        dma(out=ones_row, in_=ones_row_d[:, :])
        ones_col_bf = singles.tile([128, 1], BF16)
        dma(out=ones_col_bf, in_=ones_col_bf_d[:, :])

        eps_den = singles.tile([128, 1], F32)
        nc.vector.memset(eps_den, EPS)
        magic_i = singles.tile([128, NT8], mybir.dt.int32)
        nc.vector.memset(magic_i, 0x5F3759DF)

        inb_row = brow[:, L * 3 * 512: L * 3 * 512 + 512]
        ccb_row = brow[:, L * 3 * 512 + 512: L * 3 * 512 + 1024]
        outb_row = brow[:, L * 3 * 512 + 1024: L * 3 * 512 + 1024 + COUT]

        def brow_w(i, w):
            # w: 0=bv 1=bo 2=c2b
            return brow[:, (i * 3 + w) * 512:(i * 3 + w) * 512 + 512]

        def bcol_w(i, which, kt):
            # which: 0=bq 1=bk 2=c1b
            c = (i * 3 + which) * KT + kt
            return bcol[:, c:c + 1]

        # ---- persistent state ----
        lat = persist.tile([128, NT8 * 512], F32)
        P_all = persist.tile([128, (NCH + 1) * KT * SKW], BF16)
        nc.vector.memset(P_all[:, 0:KT * SKW], 0.0)
        # feature-tile-major activations: [within-tile row, tile, seq col]
        hT_all = persist.tile([128, KT, SH // 4 * 4], BF16)   # [e%128, kt, s]
        kT_all = persist.tile([128, KT, SH // 4 * 4], BF16)   # [feat%128, ft, t]
        h2T_all = persist.tile([128, KT, SH // 4 * 4], BF16)
        v_all = persist.tile([128, NCH * CT * 512], BF16)
        R_sb = persist.tile([128, KT * SKW], BF16)
        mR = persist.tile([128, KT * SKW], BF16)

        def Pb(j, kt):
            o = (j * KT + kt) * SKW
            return P_all[:, o:o + SKW]

        # =========== input projection + causal conv (scoped pool) ===========
        with tc.tile_pool(name="convp", bufs=1) as convp:
            x_sb = convp.tile([CIN, SH + 2], BF16)
            dma(out=x_sb, in_=x_d[:, :])
            inWT = convp.tile([CIN, E], BF16)
            dma(out=inWT, in_=inWT_d[:, :])
            ccWT = convp.tile([128, 3 * KT * 512], BF16)
            dma(out=ccWT, in_=ccWT_d[:, :])
            z = convp.tile([128, KT * (SH + 2)], BF16)
            for kt in range(KT):
                for s0, ns in ((0, 512), (512, 512), (1024, 2)):
                    pz = psum.tile([128, 512], F32, tag="one", bufs=3)
                    nc.tensor.matmul(pz[:, :ns],
                                     inWT[:, kt * 128:(kt + 1) * 128],
                                     x_sb[:, s0:s0 + ns], start=True,
                                     stop=not with_bias)
                    if with_bias:
                        nc.tensor.matmul(pz[:, :ns],
                                         inb_row[:, kt * 128:(kt + 1) * 128],
                                         halo[:, s0:s0 + ns], start=False, stop=True)
                    nc.scalar.copy(out=z[:, kt * (SH + 2) + s0: kt * (SH + 2) + s0 + ns],
                                   in_=pz[:, :ns])

            for ct8 in range(NT8):
                pc = psum.tile([128, 512], F32, tag="one", bufs=3)
                first = True
                for d in range(3):
                    for kt in range(KT):
                        zo = kt * (SH + 2) + ct8 * 128 + d
                        nc.tensor.matmul(pc[:, :],
                                         z[:, zo:zo + 128],
                                         ccWT[:, (d * KT + kt) * 512:(d * KT + kt) * 512 + 512],
                                         start=first,
                                         stop=(not with_bias and d == 2 and kt == KT - 1))
                        first = False
                if with_bias:
                    nc.tensor.matmul(pc[:, :], ones_row[:, 0:128], ccb_row,
                                     start=False, stop=True)
                nc.scalar.copy(out=lat[:, ct8 * 512:(ct8 + 1) * 512], in_=pc[:, :])

        # layer-loop pools (opened after conv pool closes)
        wq_pool = ctx.enter_context(tc.tile_pool(name="wq_pool", bufs=2))
        wo_pool = ctx.enter_context(tc.tile_pool(name="wo_pool", bufs=1))
        work = ctx.enter_context(tc.tile_pool(name="work", bufs=2))

        # =========== transformer layers ===========
        def ln_stats(mv8, ct8, sl):
            """bn stats of lat c-tile ct8 into mv8[:, 2*sl:2*sl+2]."""
            stats = small.tile([128, 6], F32, tag="lnst")
            nc.vector.bn_stats(out=stats, in_=lat[:, ct8 * 512:(ct8 + 1) * 512])
            nc.vector.bn_aggr(out=mv8[:, 2 * sl:2 * sl + 2], in_=stats)

        def ln_scales(mv8, n, tag):
            """From interleaved [mean,var] pairs build negm [128,n] and
            rstd [128,n] (Newton rsqrt on DVE; no ACT table involved)."""
            negm = small.tile([128, n], F32, tag=tag + "nm")
            var = small.tile([128, n], F32, tag=tag + "va")
            rstd = small.tile([128, n], F32, tag=tag + "rs")
            tmp = small.tile([128, n], F32, tag=tag + "tm")
            mvv = mv8[:, 0:2 * n].rearrange("p (n two) -> p n two", two=2)
            nc.vector.tensor_scalar_mul(negm, mvv[:, :, 0], -1.0)
            nc.vector.tensor_scalar_add(var, mvv[:, :, 1], LN_EPS)
            vi = var.bitcast(mybir.dt.int32)
            ri = rstd.bitcast(mybir.dt.int32)
            nc.vector.tensor_scalar(ri, vi, 1, None, op0=ALU.arith_shift_right)
            nc.vector.tensor_tensor(ri, magic_i[:, 0:n], ri, op=ALU.subtract)
            for _ in range(2):
                nc.vector.tensor_mul(tmp, rstd, rstd)
                nc.vector.tensor_mul(tmp, tmp, var)
                nc.vector.tensor_scalar(tmp, tmp, -0.5, 1.5,
                                        op0=ALU.mult, op1=ALU.add)
                nc.vector.tensor_mul(rstd, rstd, tmp)
            return negm, rstd

        def ln_norm(dst_y, ct8, negm, rstd, sl):
            nc.vector.tensor_scalar(dst_y, lat[:, ct8 * 512:(ct8 + 1) * 512],
                                    negm[:, sl:sl + 1], rstd[:, sl:sl + 1],
                                    op0=ALU.add, op1=ALU.mult)

        def ln1_sweep(mv, jp):
            """Per-pair LN1 finish: rsqrt batch, normalize, transpose to hT."""
            negm, rstd = ln_scales(mv[:, jp * CT * 2: jp * CT * 2 + 8], 4, "l1")
            y = work.tile([128, 4 * 512], BF16, tag="y", bufs=1)
            for c4 in range(4):
                ln_norm(y[:, c4 * 512:(c4 + 1) * 512], jp * CT + c4,
                        negm, rstd, c4)
            for c4 in range(4):
                nc.sync.dma_start_transpose(
                    out=hT_all[:, :, jp * 256 + c4 * 128: jp * 256 + c4 * 128 + 128],
                    in_=y[:, c4 * 512:(c4 + 1) * 512])

        def pair_proj_phi(wt, w, jp, dst3, i, which):
            """Feature-major projection for a chunk pair with phi applied.
            dst3: 3D [128, KT, SH] tile written at cols [jp*256, jp*256+512)."""
            for fh in range(2):
                pp = psum.tile([128, 1024], F32, tag="big2", bufs=2)
                for fi in range(2):
                    ft = fh * 2 + fi
                    for kt in range(KT):
                        nc.tensor.matmul(
                            pp[:, fi * 512:(fi + 1) * 512],
                            wt[:, (w * KT + kt) * 512 + ft * 128:(w * KT + kt) * 512 + ft * 128 + 128],
                            hT_all[:, kt, jp * 256: jp * 256 + 512],
                            start=(kt == 0), stop=(kt == KT - 1))
                et = work.tile([128, 1024], BF16, tag="phiE")
                for fi in range(2):
                    ft = fh * 2 + fi
                    bc = bcol_w(i, which, ft)
                    nc.scalar.activation(out=et[:, fi * 512:(fi + 1) * 512],
                                         in_=pp[:, fi * 512:(fi + 1) * 512],
                                         func=AF.Exp, bias=bc, scale=1.0)
                    nc.scalar.activation(out=dst3[:, ft, jp * 256: jp * 256 + 512],
                                         in_=pp[:, fi * 512:(fi + 1) * 512],
                                         func=AF.Relu, bias=bc, scale=1.0)
                d = dst3[:, fh * 2: fh * 2 + 2, jp * 256: jp * 256 + 512]
                nc.vector.scalar_tensor_tensor(out=d, in0=et, scalar=1.0, in1=d,
                                               op0=ALU.min, op1=ALU.add)

        # layer-0 LN1 (later layers pipeline theirs into pass 3)
        mv1 = small.tile([128, 2 * NT8], F32, tag="mv1")
        for ct8 in range(NT8):
            ln_stats(mv1, ct8, ct8)
        for jp in range(0, NCH, 2):
            ln1_sweep(mv1, jp)

        for i_rep in range(L * repeat):
            i = i_rep % L
            wq = wq_pool.tile([128, 3 * KT * 512], BF16, tag="wq")
            wo = wo_pool.tile([128, 3 * KT * 512], BF16, tag="wo")
            for wti in range(3):
                dma(out=wq[:, wti * KT * 512:(wti + 1) * KT * 512],
                    in_=wpack_d[:, (i * 6 + wti) * KT * 512:(i * 6 + wti + 1) * KT * 512])
                dma(out=wo[:, wti * KT * 512:(wti + 1) * KT * 512],
                    in_=wpack_d[:, (i * 6 + 3 + wti) * KT * 512:(i * 6 + 4 + wti) * KT * 512])

            # ---------- pass 1: k, v, local chunk states ----------
            for jp in range(0, NCH, 2):
                pair_proj_phi(wq, 1, jp, kT_all, i, 1)

                for j in (jp, jp + 1):
                    # kseq[t%128, tt, e] = phi(k)[e, t] transposed
                    kseq = work.tile([128, CT, 512], BF16, tag="kseq", bufs=1)
                    for ft in range(KT):
                        nc.sync.dma_start_transpose(
                            out=kseq[:, :, ft * 128:(ft + 1) * 128],
                            in_=kT_all[:, ft, j * 256:(j + 1) * 256])

                    # v projection (seq-major)
                    for tt in range(CT):
                        pv = psum.tile([128, 512], F32, tag="one", bufs=3)
                        for kt in range(KT):
                            nc.tensor.matmul(
                                pv[:, :],
                                hT_all[:, kt, j * 256 + tt * 128: j * 256 + tt * 128 + 128],
                                wq[:, (2 * KT + kt) * 512:(2 * KT + kt) * 512 + 512],
                                start=(kt == 0),
                                stop=(not with_bias and kt == KT - 1))
                        if with_bias:
                            nc.tensor.matmul(pv[:, :], ones_row[:, 0:128],
                                             brow_w(i, 0), start=False, stop=True)
                        nc.scalar.copy(
                            out=v_all[:, (j * CT + tt) * 512:(j * CT + tt) * 512 + 512],
                            in_=pv[:, :])

                    # delta state + prefix chain:  P[j+1] = P[j] + kseq^T [v|1]
                    skd = small.tile([128, KT], F32, tag="skd")
                    for kt in range(KT):
                        nc.vector.reduce_sum(
                            out=skd[:, kt:kt + 1],
                            in_=kT_all[:, kt, j * 256:(j + 1) * 256],
                            axis=mybir.AxisListType.X)
                        pd = psum.tile([128, 512], F32, tag="one", bufs=3)
                        for tt in range(CT):
                            ks = kseq[:, tt, kt * 128:(kt + 1) * 128]
                            nc.tensor.matmul(
                                pd[:, :], ks,
                                v_all[:, (j * CT + tt) * 512:(j * CT + tt) * 512 + 512],
                                start=(tt == 0), stop=(tt == CT - 1))
                        nc.vector.scalar_tensor_tensor(
                            out=Pb(j + 1, kt)[:, 0:E], in0=pd[:, :], scalar=1.0,
                            in1=Pb(j, kt)[:, 0:E], op0=ALU.mult, op1=ALU.add)
                    for kt in range(KT):
                        nc.vector.scalar_tensor_tensor(
                            out=Pb(j + 1, kt)[:, E:SKW], in0=skd[:, kt:kt + 1], scalar=1.0,
                            in1=Pb(j, kt)[:, E:SKW], op0=ALU.mult, op1=ALU.add)

            # ---------- boundary-state exchange ----------
            contrib = work.tile([128, KT * SKW], BF16, tag="contrib", bufs=1)
            nc.vector.tensor_scalar_mul(contrib,
                                        P_all[:, NCH * KT * SKW:(NCH + 1) * KT * SKW],
                                        mcol[:, 1:2])
            cc_out = dram.tile([128, KT * SKW], BF16, tag="cc_out")
            cc_in = dram.tile([128, KT * SKW], BF16, tag="cc_in")
            nc.gpsimd.dma_start(out=cc_out, in_=contrib)
            if cc:
                nc.gpsimd.collective_compute(
                    "AllReduce", ALU.add, replica_groups=REPLICA_GROUPS,
                    ins=[cc_out.opt()], outs=[cc_in.opt()])
            else:
                nc.gpsimd.dma_start(out=cc_in.opt(), in_=cc_out.opt())
            nc.gpsimd.dma_start(out=R_sb, in_=cc_in)
            nc.vector.tensor_scalar_mul(mR, R_sb, mcol[:, 0:1])

            # ---------- pass 2: attention output + LN2 ----------
            mv2 = small.tile([128, 2 * NT8], F32, tag="mv2")
            for jp in range(0, NCH, 2):
                qTp = work.tile([128, KT, 512], BF16, tag="qT")
                # q projection+phi for the pair (reuse pair machinery on a
                # pair-local 3D tile: cols [0,512) correspond to jp..jp+1)
                for fh in range(2):
                    pp = psum.tile([128, 1024], F32, tag="big2", bufs=2)
                    for fi in range(2):
                        ft = fh * 2 + fi
                        for kt in range(KT):
                            nc.tensor.matmul(
                                pp[:, fi * 512:(fi + 1) * 512],
                                wq[:, (0 * KT + kt) * 512 + ft * 128:(0 * KT + kt) * 512 + ft * 128 + 128],
                                hT_all[:, kt, jp * 256: jp * 256 + 512],
                                start=(kt == 0), stop=(kt == KT - 1))
                    et = work.tile([128, 1024], BF16, tag="phiE")
                    for fi in range(2):
                        ft = fh * 2 + fi
                        bc = bcol_w(i, 0, ft)
                        nc.scalar.activation(out=et[:, fi * 512:(fi + 1) * 512],
                                             in_=pp[:, fi * 512:(fi + 1) * 512],
                                             func=AF.Exp, bias=bc, scale=1.0)
                        nc.scalar.activation(out=qTp[:, ft, :],
                                             in_=pp[:, fi * 512:(fi + 1) * 512],
                                             func=AF.Relu, bias=bc, scale=1.0)
                    d = qTp[:, fh * 2: fh * 2 + 2, :]
                    nc.vector.scalar_tensor_tensor(out=d, in0=et, scalar=1.0,
                                                   in1=d, op0=ALU.min, op1=ALU.add)

                for j in (jp, jp + 1):
                    jo = (j & 1) * 256
                    if j == 0:
                        Peff = mR  # exclusive prefix of chunk 0 is zero
                    else:
                        Peff = work.tile([128, KT * SKW], BF16, tag="Peff")
                        nc.vector.scalar_tensor_tensor(
                            out=Peff, in0=P_all[:, j * KT * SKW:(j + 1) * KT * SKW],
                            scalar=1.0, in1=mR, op0=ALU.mult, op1=ALU.add)

                    # scoresT: cols 0:256 = t0 x (s0|s1); cols 256:384 = t1 x s1
                    ps = psum.tile([128, 384], F32, tag="sm", bufs=1)
                    for ft in range(KT):
                        nc.tensor.matmul(
                            ps[:, 0:256],
                            kT_all[:, ft, j * 256: j * 256 + 128],
                            qTp[:, ft, jo: jo + 256],
                            start=(ft == 0), stop=False)
                        nc.tensor.matmul(
                            ps[:, 256:384],
                            kT_all[:, ft, j * 256 + 128: j * 256 + 256],
                            qTp[:, ft, jo + 128: jo + 256],
                            start=False, stop=(ft == KT - 1))
                    sm = work.tile([128, 384], BF16, tag="sm")
                    nc.vector.tensor_mul(sm[:, 0:128], ps[:, 0:128], tril)
                    nc.scalar.copy(out=sm[:, 128:256], in_=ps[:, 128:256])
                    nc.vector.tensor_mul(sm[:, 256:384], ps[:, 256:384], tril)

                    # num, seq-major: [s, e'] = masked-scores @ v + q @ KV.
                    # intra: lhsT = sm blocks [t, s]; inter: lhsT = qT slices.
                    pn0 = psum.tile([128, 512], F32, tag="one", bufs=3)
                    pn1 = psum.tile([128, 512], F32, tag="one", bufs=3)
                    v0 = v_all[:, (j * CT + 0) * 512:(j * CT + 0) * 512 + 512]
                    v1 = v_all[:, (j * CT + 1) * 512:(j * CT + 1) * 512 + 512]
                    nc.tensor.matmul(pn0[:, :], sm[:, 0:128], v0,
                                     start=True, stop=False)
                    nc.tensor.matmul(pn1[:, :], sm[:, 128:256], v0,
                                     start=True, stop=False)
                    nc.tensor.matmul(pn1[:, :], sm[:, 256:384], v1,
                                     start=False, stop=False)
                    for kt in range(KT):
                        nc.tensor.matmul(pn0[:, :],
                                         qTp[:, kt, jo: jo + 128],
                                         Peff[:, kt * SKW: kt * SKW + E],
                                         start=False, stop=(kt == KT - 1))
                        nc.tensor.matmul(pn1[:, :],
                                         qTp[:, kt, jo + 128: jo + 256],
                                         Peff[:, kt * SKW: kt * SKW + E],
                                         start=False, stop=(kt == KT - 1))

                    # den: column sums of masked scores + q . s_k
                    pden = psum.tile([128, CT], F32, tag="sm", bufs=1)
                    nc.tensor.matmul(pden[:, 0:1], sm[:, 0:128], ones_col_bf,
                                     start=True, stop=False)
                    nc.tensor.matmul(pden[:, 1:2], sm[:, 128:256], ones_col_bf,
                                     start=False, stop=False)
                    nc.tensor.matmul(pden[:, 1:2], sm[:, 256:384], ones_col_bf,
                                     start=False, stop=False)
                    for st in range(CT):
                        for kt in range(KT):
                            nc.tensor.matmul(
                                pden[:, st:st + 1],
                                qTp[:, kt, jo + st * 128: jo + st * 128 + 128],
                                Peff[:, kt * SKW + E: kt * SKW + SKW],
                                start=False, stop=(st == CT - 1 and kt == KT - 1))
                    den = small.tile([128, CT], F32, tag="den")
                    nc.scalar.activation(out=den, in_=pden[:, :], func=AF.Identity,
                                         bias=eps_den, scale=1.0)
                    rden = small.tile([128, CT], F32, tag="rden")
                    nc.vector.reciprocal(out=rden, in_=den)

                    # attn = num/den (seq-major, natural per-partition scale),
                    # then DMA-transpose to feature-major for the o-projection
                    attn = work.tile([128, CT * 512], BF16, tag="numT")
                    nc.scalar.activation(out=attn[:, 0:512], in_=pn0[:, :],
                                         func=AF.Copy, scale=rden[:, 0:1])
                    nc.scalar.activation(out=attn[:, 512:1024], in_=pn1[:, :],
                                         func=AF.Copy, scale=rden[:, 1:2])
                    attnT = work.tile([128, KT, 256], BF16, tag="attnT")
                    for st in range(CT):
                        nc.sync.dma_start_transpose(
                            out=attnT[:, :, st * 128:(st + 1) * 128],
                            in_=attn[:, st * 512:(st + 1) * 512])

                    # o-projection + residual
                    for st in range(CT):
                        po = psum.tile([128, 512], F32, tag="one", bufs=3)
                        for mt in range(KT):
                            nc.tensor.matmul(po[:, :],
                                             attnT[:, mt, st * 128:(st + 1) * 128],
                                             wo[:, (0 * KT + mt) * 512:(0 * KT + mt) * 512 + 512],
                                             start=(mt == 0),
                                             stop=(not with_bias and mt == KT - 1))
                        if with_bias:
                            nc.tensor.matmul(po[:, :], ones_row[:, 0:128],
                                             brow_w(i, 1), start=False, stop=True)
                        ls = lat[:, (j * CT + st) * 512:(j * CT + st) * 512 + 512]
                        nc.vector.scalar_tensor_tensor(out=ls, in0=po[:, :],
                                                       scalar=1.0,
                                                       in1=ls, op0=ALU.mult, op1=ALU.add)

                    # LN2 stats here (post-residual); scales batched per pair
                    for ct in range(CT):
                        ln_stats(mv2, j * CT + ct, j * CT + ct)

                # LN2 normalize + transpose sweep for this pair (DVE + DMA)
                negm2, rstd2 = ln_scales(mv2[:, jp * CT * 2: jp * CT * 2 + 8],
                                         4, "l2")
                for c4 in range(4):
                    ct8 = jp * CT + c4
                    y2 = work.tile([128, 512], BF16, tag="y2", bufs=2)
                    ln_norm(y2, ct8, negm2, rstd2, c4)
                    nc.sync.dma_start_transpose(
                        out=h2T_all[:, :, ct8 * 128:(ct8 + 1) * 128],
                        in_=y2)

            # ---------- pass 3: FFN (+ next layer's LN1, pipelined) ----------
            mv1n = small.tile([128, 2 * NT8], F32, tag="mv1")
            for jp in range(0, NCH, 2):
                h1T = work.tile([128, KT, 512], BF16, tag="h1T", bufs=1)
                for fh in range(2):
                    ph1 = psum.tile([128, 1024], F32, tag="big2", bufs=2)
                    for fi in range(2):
                        ft = fh * 2 + fi
                        for kt in range(KT):
                            nc.tensor.matmul(
                                ph1[:, fi * 512:(fi + 1) * 512],
                                wo[:, (1 * KT + kt) * 512 + ft * 128:(1 * KT + kt) * 512 + ft * 128 + 128],
                                h2T_all[:, kt, jp * 256: jp * 256 + 512],
                                start=(kt == 0), stop=(kt == KT - 1))
                    for fi in range(2):
                        ft = fh * 2 + fi
                        nc.scalar.activation(out=h1T[:, ft, :],
                                             in_=ph1[:, fi * 512:(fi + 1) * 512],
                                             func=AF.Gelu, bias=bcol_w(i, 2, ft),
                                             scale=1.0)

                for j in (jp, jp + 1):
                    jo = (j & 1) * 256
                    for st in range(CT):
                        pf = psum.tile([128, 512], F32, tag="one", bufs=3)
                        for mt in range(KT):
                            nc.tensor.matmul(
                                pf[:, :],
                                h1T[:, mt, jo + st * 128: jo + st * 128 + 128],
                                wo[:, (2 * KT + mt) * 512:(2 * KT + mt) * 512 + 512],
                                start=(mt == 0),
                                stop=(not with_bias and mt == KT - 1))
                        if with_bias:
                            nc.tensor.matmul(pf[:, :], ones_row[:, 0:128],
                                             brow_w(i, 2), start=False, stop=True)
                        ls = lat[:, (j * CT + st) * 512:(j * CT + st) * 512 + 512]
                        nc.vector.scalar_tensor_tensor(out=ls, in0=pf[:, :], scalar=1.0,
                                                       in1=ls, op0=ALU.mult, op1=ALU.add)
                        if i_rep < L * repeat - 1:
                            ln_stats(mv1n, j * CT + st, j * CT + st)
                if i_rep < L * repeat - 1:
                    ln1_sweep(mv1n, jp)

        # =========== output projection ===========
        with tc.tile_pool(name="tail", bufs=1) as tailp:
            latT = tailp.tile([128, KT, SH], BF16)
            for ct8 in range(NT8):
                latb = work.tile([128, 512], BF16, tag="y2")
                nc.vector.tensor_copy(out=latb, in_=lat[:, ct8 * 512:(ct8 + 1) * 512])
                nc.sync.dma_start_transpose(
                    out=latT[:, :, ct8 * 128:(ct8 + 1) * 128], in_=latb)

            out_sb = tailp.tile([COUT, SH], F32)
            for sb in range(SH // 512):
                pout = psum.tile([COUT, 512], F32, tag="one", bufs=3)
                for kt in range(KT):
                    nc.tensor.matmul(pout[:, :],
                                     outWT[:, kt * COUT:(kt + 1) * COUT],
                                     latT[:, kt, sb * 512:(sb + 1) * 512],
                                     start=(kt == 0),
                                     stop=(not with_bias and kt == KT - 1))
                if with_bias:
                    nc.tensor.matmul(pout[:, :], outb_row, ones_row,
                                     start=False, stop=True)
                nc.scalar.copy(out=out_sb[:, sb * 512:(sb + 1) * 512], in_=pout[:, :])
            dma(out=out_d[:, :], in_=out_sb)


# ---------------- host side ----------------

_CACHE = threading.local()


def _get_program(with_bias=False):
    key = f"nc_{with_bias}"
    if not hasattr(_CACHE, key):
        setattr(_CACHE, key, build_program(with_bias=with_bias))
    return getattr(_CACHE, key)


def _needs_bias(inputs):
    f32 = np.float32
    ln1_b = np.asarray(inputs["ln1_b"], f32)
    ln2_b = np.asarray(inputs["ln2_b"], f32)
    vals = [np.asarray(inputs[k], f32) for k in
            ("in_b", "cc_b", "out_b", "bo", "c2_b")]
    bv_eff = np.asarray(inputs["bv"], f32) + np.einsum(
        "loe,le->lo", np.asarray(inputs["Wv"], f32), ln1_b)
    vals.append(bv_eff)
    return any(np.abs(v).max() > 0 for v in vals)


def _prep_shared(inputs):
    f32 = np.float32
    inW = np.asarray(inputs["in_W"], f32)      # [E, CIN]
    in_b = np.asarray(inputs["in_b"], f32)
    ccW = np.asarray(inputs["cc_W"], f32)      # [E, E, 3]
    cc_b = np.asarray(inputs["cc_b"], f32)
    outW = np.asarray(inputs["out_W"], f32)    # [COUT, E]
    out_b = np.asarray(inputs["out_b"], f32)

    ccWT = np.zeros((128, 3 * KT * 512), f32)
    for d in range(3):
        WT = ccW[:, :, d].T  # [e_in, e_out]
        for kt in range(KT):
            ccWT[:, (d * KT + kt) * 512:(d * KT + kt) * 512 + 512] = \
                WT[kt * 128:(kt + 1) * 128, :]

    ln1_g = np.asarray(inputs["ln1_g"], f32); ln1_b = np.asarray(inputs["ln1_b"], f32)
    ln2_g = np.asarray(inputs["ln2_g"], f32); ln2_b = np.asarray(inputs["ln2_b"], f32)

    wpack = np.zeros((128, L * 6 * KT * 512), f32)
    brow = np.zeros((1, BROW_N), f32)
    bcol = np.zeros((128, L * 3 * KT), f32)
    for i in range(L):
        biases = {}
        for w, (Wn, bn, g, bb) in enumerate((
                ("Wq", "bq", ln1_g[i], ln1_b[i]),
                ("Wk", "bk", ln1_g[i], ln1_b[i]),
                ("Wv", "bv", ln1_g[i], ln1_b[i]),
                ("Wo", "bo", None, None),
                ("c1_W", "c1_b", ln2_g[i], ln2_b[i]),
                ("c2_W", "c2_b", None, None))):
            W = np.asarray(inputs[Wn], f32)[i]          # [E_out, E_in]
            bias = np.asarray(inputs[bn], f32)[i].copy()
            if g is not None:
                WT = (W * g[None, :]).T                  # fold LN gain
                bias = bias + W @ bb                     # fold LN bias
            else:
                WT = W.T
            for kt in range(KT):
                wpack[:, (i * 6 + w) * KT * 512 + kt * 512:
                      (i * 6 + w) * KT * 512 + kt * 512 + 512] = \
                    WT[kt * 128:(kt + 1) * 128, :]
            biases[w] = bias
        # rows: bv, bo, c2b
        brow[0, (i * 3 + 0) * 512:(i * 3 + 0) * 512 + 512] = biases[2]
        brow[0, (i * 3 + 1) * 512:(i * 3 + 1) * 512 + 512] = biases[3]
        brow[0, (i * 3 + 2) * 512:(i * 3 + 2) * 512 + 512] = biases[5]
        # cols: bq, bk, c1b
        for which, w in ((0, 0), (1, 1), (2, 4)):
            for kt in range(KT):
                bcol[:, (i * 3 + which) * KT + kt] = biases[w][kt * 128:(kt + 1) * 128]

    inWT = inW.T  # [CIN, E]
    outWT = np.zeros((128, KT * COUT), f32)
    for kt in range(KT):
        outWT[:, kt * COUT:(kt + 1) * COUT] = outW.T[kt * 128:(kt + 1) * 128, :]

    brow[0, L * 3 * 512: L * 3 * 512 + 512] = in_b
    brow[0, L * 3 * 512 + 512: L * 3 * 512 + 1024] = cc_b
    brow[0, L * 3 * 512 + 1024: L * 3 * 512 + 1024 + COUT] = out_b

    tril = np.tril(np.ones((128, 128), f32)).T  # keep t<=s in [t,s] layout

    return {
        "inWT": inWT.astype(BF),
        "ccWT": ccWT.astype(BF),
        "wpack": wpack.astype(BF),
        "outWT": outWT.astype(BF),
        "brow": brow.astype(BF),
        "bcol": bcol,
        "tril": tril.astype(BF),
        "ones_row": np.ones((1, 512), f32).astype(BF),
        "ones_col_bf": np.ones((128, 1), f32).astype(BF),
    }


def _prep_core_inputs(shared, inputs, b, h):
    f32 = np.float32
    x = np.asarray(inputs["x"], f32)
    s0 = h * SH
    x_sl = np.zeros((CIN, SH + 2), f32)
    lo = max(0, s0 - 2)
    x_sl[:, 2 - (s0 - lo):] = x[b, :, lo:s0 + SH]
    halo = np.ones((1, SH + 2), f32)
    if h == 0:
        halo[0, :2] = 0.0
    mcol = np.zeros((128, 2), f32)
    mcol[:, 0] = float(h)
    mcol[:, 1] = 1.0 - float(h)
    m = dict(shared)
    m["x_sl"] = x_sl.astype(BF)
    m["halo"] = halo.astype(BF)
    m["mcol"] = mcol
    return m


def _run(inputs, **kw):
    nc = _get_program(with_bias=_needs_bias(inputs))
    shared = _prep_shared(inputs)
    in_maps = []
    for core in range(NCORES):
        b, h = core // 2, core % 2
        in_maps.append(_prep_core_inputs(shared, inputs, b, h))
    return run_bass_kernel_spmd(nc, in_maps, core_ids=list(range(NCORES)), **kw)


def kernel(**inputs):
    res = _run(inputs)
    out = np.zeros((B, COUT, S), np.float32)
    for core in range(NCORES):
        b, h = core // 2, core % 2
        out[b, :, h * SH:(h + 1) * SH] = res.results[core]["out"]
    return out


def bench(inputs, trace_cores=(0, 1), tmpdir=None):
    """Run with NTFF tracing; returns BassKernelResults with exec_time_ns."""
    return _run(inputs, trace=True, trace_cores=list(trace_cores), tmpdir=tmpdir)



# revision 12
# speedup vs baseline: 128.0994x; 128.0994x over previous
"""Recurrent linear-attention transformer on 8 Trainium2 NeuronCores.

Sharding: 4-way data parallel over batch x 2-way sequence split (halves of
S=2048). Causal linear attention is computed in chunked form; the only
cross-core dependency is the cumulative (k^T v, sum k) state at the half
boundary, exchanged once per layer via a pairwise AllReduce.

Per-core layout conventions (SBUF 2-D tensors, 128 partitions):
  lat   f32  [128, 8*512]    seq-major residual: c-tile ct8 block, cols = e
  hT    bf16 [128, 4*4*256]  feature-major LN1 output: (chunk, kt) blocks
  kT    bf16 [128, 4*4*256]  phi(k) feature-major: (chunk, ft) blocks, cols=t
  v     bf16 [128, 4*2*512]  v seq-major: (chunk, tt) blocks, cols = e'
  P_all bf16 [128, 5*4*513]  prefix states: (j, kt) -> [KV[e,e'] | s_k[e]]
All matmul operands bf16, PSUM/stats/residual f32.
"""

import threading

import numpy as np
import ml_dtypes

import concourse.bass as bass
import concourse.bacc as bacc
import concourse.tile as tile
import concourse.mybir as mybir
from concourse.bass_utils import run_bass_kernel_spmd

AF = mybir.ActivationFunctionType
ALU = mybir.AluOpType
F32 = mybir.dt.float32
BF16 = mybir.dt.bfloat16
BF = ml_dtypes.bfloat16

L, B, CIN, COUT, E, S = 4, 4, 64, 64, 512, 2048
NCORES = 8
SH = S // 2          # per-core sequence half
C = 256              # attention chunk
NCH = SH // C        # 4 chunks
CT = C // 128        # 2 c-tiles per chunk
KT = E // 128        # 4 feature tiles
NT8 = SH // 128      # 8 seq tiles per half
EPS = 1e-6
LN_EPS = 1e-5
SKW = E + 1          # 513: KV block plus s_k column

REPLICA_GROUPS = [[0, 1], [2, 3], [4, 5], [6, 7]]

# brow layout (bf16): per-layer [bv, bo, c2b] rows, then in_b, cc_b, out_b
BROW_N = L * 3 * 512 + 512 + 512 + COUT


def build_program(cc=True, repeat=1, with_bias=False, body_repeat=1):
    nc = bacc.Bacc("TRN2", target_bir_lowering=False, debug=False,
                   num_devices=NCORES)

    x_d = nc.dram_tensor("x_sl", [CIN, SH + 2], BF16, kind="ExternalInput")
    inWT_d = nc.dram_tensor("inWT", [CIN, E], BF16, kind="ExternalInput")
    ccWT_d = nc.dram_tensor("ccWT", [128, 3 * KT * 512], BF16, kind="ExternalInput")
    wpack_d = nc.dram_tensor("wpack", [128, L * 6 * KT * 512], BF16, kind="ExternalInput")
    outWT_d = nc.dram_tensor("outWT", [128, KT * COUT], BF16, kind="ExternalInput")
    brow_d = nc.dram_tensor("brow", [1, BROW_N], BF16, kind="ExternalInput")
    bcol_d = nc.dram_tensor("bcol", [128, L * 3 * KT], F32, kind="ExternalInput")
    tril_d = nc.dram_tensor("tril", [128, 128], BF16, kind="ExternalInput")
    mcol_d = nc.dram_tensor("mcol", [128, 2], F32, kind="ExternalInput")  # [m, 1-m]
    halo_d = nc.dram_tensor("halo", [1, SH + 2], BF16, kind="ExternalInput")
    ones_row_d = nc.dram_tensor("ones_row", [1, 512], BF16, kind="ExternalInput")
    ones_col_bf_d = nc.dram_tensor("ones_col_bf", [128, 1], BF16, kind="ExternalInput")

    out_d = nc.dram_tensor("out", [COUT, SH], F32, kind="ExternalOutput")

    with tile.TileContext(nc, num_cores=NCORES) as tc:
        for _body in range(body_repeat):
            _emit(nc, tc, x_d, inWT_d, ccWT_d, wpack_d, outWT_d, brow_d, bcol_d,
                  tril_d, mcol_d, halo_d, ones_row_d,
                  ones_col_bf_d, out_d, cc=cc, repeat=repeat,
                  with_bias=with_bias)
    nc.compile()
    return nc


def _emit(nc, tc, x_d, inWT_d, ccWT_d, wpack_d, outWT_d, brow_d, bcol_d,
          tril_d, mcol_d, halo_d, ones_row_d,
          ones_col_bf_d, out_d, cc=True, repeat=1,
          with_bias=False):
    import contextlib
    ctx = contextlib.ExitStack()
    with ctx:
        singles = ctx.enter_context(tc.tile_pool(name="singles", bufs=1))
        persist = ctx.enter_context(tc.tile_pool(name="persist", bufs=1))
        small = ctx.enter_context(tc.tile_pool(name="small", bufs=4))
        psum = ctx.enter_context(tc.tile_pool(name="psum", bufs=2, space="PSUM"))
        dram = ctx.enter_context(tc.tile_pool(name="dram", bufs=2, space="DRAM"))

        dma = nc.sync.dma_start

        # ---- constants ----
        outWT = singles.tile([128, KT * COUT], BF16)
        dma(out=outWT, in_=outWT_d[:, :])
        brow = singles.tile([1, BROW_N], BF16)
        dma(out=brow, in_=brow_d[:, :])
        bcol = singles.tile([128, L * 3 * KT], F32)
        dma(out=bcol, in_=bcol_d[:, :])
        tril = singles.tile([128, 128], BF16)
        dma(out=tril, in_=tril_d[:, :])
        mcol = singles.tile([128, 2], F32)
        dma(out=mcol, in_=mcol_d[:, :])
        halo = singles.tile([1, SH + 2], BF16)
        dma(out=halo, in_=halo_d[:, :])
        ones_row = singles.tile([1, 512], BF16)
        dma(out=ones_row, in_=ones_row_d[:, :])
        ones_col_bf = singles.tile([128, 1], BF16)
        dma(out=ones_col_bf, in_=ones_col_bf_d[:, :])

        eps_den = singles.tile([128, 1], F32)
        nc.vector.memset(eps_den, EPS)
        magic_i = singles.tile([128, NT8], mybir.dt.int32)
        nc.vector.memset(magic_i, 0x5F3759DF)

        inb_row = brow[:, L * 3 * 512: L * 3 * 512 + 512]
        ccb_row = brow[:, L * 3 * 512 + 512: L * 3 * 512 + 1024]
        outb_row = brow[:, L * 3 * 512 + 1024: L * 3 * 512 + 1024 + COUT]

        def brow_w(i, w):
            # w: 0=bv 1=bo 2=c2b
            return brow[:, (i * 3 + w) * 512:(i * 3 + w) * 512 + 512]

        def bcol_w(i, which, kt):
            # which: 0=bq 1=bk 2=c1b
            c = (i * 3 + which) * KT + kt
            return bcol[:, c:c + 1]

        # ---- persistent state ----
        lat = persist.tile([128, NT8 * 512], F32)
        P_all = persist.tile([128, (NCH + 1) * KT * SKW], BF16)
        nc.vector.memset(P_all[:, 0:KT * SKW], 0.0)
        # feature-tile-major activations: [within-tile row, tile, seq col]
        hT_all = persist.tile([128, KT, SH // 4 * 4], BF16)   # [e%128, kt, s]
        kT_all = persist.tile([128, KT, SH // 4 * 4], BF16)   # [feat%128, ft, t]
        h2T_all = persist.tile([128, KT, SH // 4 * 4], BF16)
        v_all = persist.tile([128, NCH * CT * 512], BF16)
        mR = persist.tile([128, KT * SKW], BF16)

        def Pb(j, kt):
            o = (j * KT + kt) * SKW
            return P_all[:, o:o + SKW]

        # =========== input projection + causal conv (scoped pool) ===========
        with tc.tile_pool(name="convp", bufs=1) as convp:
            x_sb = convp.tile([CIN, SH + 2], BF16)
            dma(out=x_sb, in_=x_d[:, :])
            inWT = convp.tile([CIN, E], BF16)
            dma(out=inWT, in_=inWT_d[:, :])
            ccWT = convp.tile([128, 3 * KT * 512], BF16)
            dma(out=ccWT, in_=ccWT_d[:, :])
            z = convp.tile([128, KT * (SH + 2)], BF16)
            for kt in range(KT):
                for s0, ns in ((0, 512), (512, 512), (1024, 2)):
                    pz = psum.tile([128, 512], F32, tag="one", bufs=4)
                    nc.tensor.matmul(pz[:, :ns],
                                     inWT[:, kt * 128:(kt + 1) * 128],
                                     x_sb[:, s0:s0 + ns], start=True,
                                     stop=not with_bias)
                    if with_bias:
                        nc.tensor.matmul(pz[:, :ns],
                                         inb_row[:, kt * 128:(kt + 1) * 128],
                                         halo[:, s0:s0 + ns], start=False, stop=True)
                    nc.scalar.copy(out=z[:, kt * (SH + 2) + s0: kt * (SH + 2) + s0 + ns],
                                   in_=pz[:, :ns])

            for ct8 in range(NT8):
                pc = psum.tile([128, 512], F32, tag="one", bufs=4)
                first = True
                for d in range(3):
                    for kt in range(KT):
                        zo = kt * (SH + 2) + ct8 * 128 + d
                        nc.tensor.matmul(pc[:, :],
                                         z[:, zo:zo + 128],
                                         ccWT[:, (d * KT + kt) * 512:(d * KT + kt) * 512 + 512],
                                         start=first,
                                         stop=(not with_bias and d == 2 and kt == KT - 1))
                        first = False
                if with_bias:
                    nc.tensor.matmul(pc[:, :], ones_row[:, 0:128], ccb_row,
                                     start=False, stop=True)
                nc.scalar.copy(out=lat[:, ct8 * 512:(ct8 + 1) * 512], in_=pc[:, :])

        # layer-loop pools (opened after conv pool closes)
        wq_pool = ctx.enter_context(tc.tile_pool(name="wq_pool", bufs=2))
        wo_pool = ctx.enter_context(tc.tile_pool(name="wo_pool", bufs=2))
        work = ctx.enter_context(tc.tile_pool(name="work", bufs=2))

        # =========== transformer layers ===========
        def ln_stats(mv8, ct8, sl):
            """bn stats of lat c-tile ct8 into mv8[:, 2*sl:2*sl+2]."""
            stats = small.tile([128, 6], F32, tag="lnst")
            nc.vector.bn_stats(out=stats, in_=lat[:, ct8 * 512:(ct8 + 1) * 512])
            nc.vector.bn_aggr(out=mv8[:, 2 * sl:2 * sl + 2], in_=stats)

        def ln_scales(mv8, n, tag):
            """From interleaved [mean,var] pairs build nmr=-mean*rstd [128,n]
            and rstd [128,n] (Newton rsqrt on DVE; no ACT table involved)."""
            negm = small.tile([128, n], F32, tag=tag + "nm")
            var = small.tile([128, n], F32, tag=tag + "va")
            rstd = small.tile([128, n], F32, tag=tag + "rs")
            tmp = small.tile([128, n], F32, tag=tag + "tm")
            nmr = small.tile([128, n], F32, tag=tag + "nmr")
            mvv = mv8[:, 0:2 * n].rearrange("p (n two) -> p n two", two=2)
            nc.vector.tensor_scalar_mul(negm, mvv[:, :, 0], -1.0)
            nc.vector.tensor_scalar_add(var, mvv[:, :, 1], LN_EPS)
            vi = var.bitcast(mybir.dt.int32)
            ri = rstd.bitcast(mybir.dt.int32)
            nc.vector.tensor_scalar(ri, vi, 1, None, op0=ALU.arith_shift_right)
            nc.vector.tensor_tensor(ri, magic_i[:, 0:n], ri, op=ALU.subtract)
            for _ in range(2):
                nc.vector.tensor_mul(tmp, rstd, rstd)
                nc.vector.tensor_mul(tmp, tmp, var)
                nc.vector.tensor_scalar(tmp, tmp, -0.5, 1.5,
                                        op0=ALU.mult, op1=ALU.add)
                nc.vector.tensor_mul(rstd, rstd, tmp)
            nc.vector.tensor_mul(nmr, negm, rstd)
            return nmr, rstd

        def ln_norm(dst_y, ct8, nmr, rstd, sl):
            # (x - m) * r as r*x + (-m*r) on DVE (ptr scalars)
            nc.vector.tensor_scalar(dst_y, lat[:, ct8 * 512:(ct8 + 1) * 512],
                                    rstd[:, sl:sl + 1], nmr[:, sl:sl + 1],
                                    op0=ALU.mult, op1=ALU.add)

        def ln1_sweep(mv, jp):
            """Per-pair LN1 finish: rsqrt batch, normalize, transpose to hT."""
            negm, rstd = ln_scales(mv[:, jp * CT * 2: jp * CT * 2 + 8], 4, "l1")
            y = work.tile([128, 4 * 512], BF16, tag="y", bufs=2)
            for c4 in range(4):
                ln_norm(y[:, c4 * 512:(c4 + 1) * 512], jp * CT + c4,
                        negm, rstd, c4)
            for c4 in range(4):
                nc.sync.dma_start_transpose(
                    out=hT_all[:, :, jp * 256 + c4 * 128: jp * 256 + c4 * 128 + 128],
                    in_=y[:, c4 * 512:(c4 + 1) * 512])

        def pair_proj_phi(wt, w, jp, dst3, i, which):
            """Feature-major projection for a chunk pair with phi applied.
            dst3: 3D [128, KT, SH] tile written at cols [jp*256, jp*256+512)."""
            for fh in range(2):
                pp = psum.tile([128, 1024], F32, tag="big2", bufs=1)
                for fi in range(2):
                    ft = fh * 2 + fi
                    for kt in range(KT):
                        nc.tensor.matmul(
                            pp[:, fi * 512:(fi + 1) * 512],
                            wt[:, (w * KT + kt) * 512 + ft * 128:(w * KT + kt) * 512 + ft * 128 + 128],
                            hT_all[:, kt, jp * 256: jp * 256 + 512],
                            start=(kt == 0), stop=(kt == KT - 1))
                et = work.tile([128, 1024], BF16, tag="phiE")
                for fi in range(2):
                    ft = fh * 2 + fi
                    bc = bcol_w(i, which, ft)
                    nc.scalar.activation(out=et[:, fi * 512:(fi + 1) * 512],
                                         in_=pp[:, fi * 512:(fi + 1) * 512],
                                         func=AF.Exp, bias=bc, scale=1.0)
                    nc.scalar.activation(out=dst3[:, ft, jp * 256: jp * 256 + 512],
                                         in_=pp[:, fi * 512:(fi + 1) * 512],
                                         func=AF.Relu, bias=bc, scale=1.0)
                d = dst3[:, fh * 2: fh * 2 + 2, jp * 256: jp * 256 + 512]
                nc.vector.scalar_tensor_tensor(out=d, in0=et, scalar=1.0, in1=d,
                                               op0=ALU.min, op1=ALU.add)

        # layer-0 LN1 (later layers pipeline theirs into pass 3)
        mv1 = small.tile([128, 2 * NT8], F32, tag="mv1")
        for ct8 in range(NT8):
            ln_stats(mv1, ct8, ct8)
        for jp in range(0, NCH, 2):
            ln1_sweep(mv1, jp)

        for i_rep in range(L * repeat):
            i = i_rep % L
            wq = wq_pool.tile([128, 3 * KT * 512], BF16, tag="wq")
            wo = wo_pool.tile([128, 3 * KT * 512], BF16, tag="wo")
            for wti in range(3):
                dma(out=wq[:, wti * KT * 512:(wti + 1) * KT * 512],
                    in_=wpack_d[:, (i * 6 + wti) * KT * 512:(i * 6 + wti + 1) * KT * 512])
                dma(out=wo[:, wti * KT * 512:(wti + 1) * KT * 512],
                    in_=wpack_d[:, (i * 6 + 3 + wti) * KT * 512:(i * 6 + 4 + wti) * KT * 512])

            # ---------- pass 1: k, v, local chunk states ----------
            for jp in range(0, NCH, 2):
                pair_proj_phi(wq, 1, jp, kT_all, i, 1)

                for j in (jp, jp + 1):
                    # kseq[t%128, tt, e] = phi(k)[e, t] transposed
                    kseq = work.tile([128, CT, 512], BF16, tag="kseq", bufs=2)
                    for ft in range(KT):
                        nc.sync.dma_start_transpose(
                            out=kseq[:, :, ft * 128:(ft + 1) * 128],
                            in_=kT_all[:, ft, j * 256:(j + 1) * 256])

                    # v projection (seq-major)
                    for tt in range(CT):
                        pv = psum.tile([128, 512], F32, tag="one", bufs=4)
                        for kt in range(KT):
                            nc.tensor.matmul(
                                pv[:, :],
                                hT_all[:, kt, j * 256 + tt * 128: j * 256 + tt * 128 + 128],
                                wq[:, (2 * KT + kt) * 512:(2 * KT + kt) * 512 + 512],
                                start=(kt == 0),
                                stop=(not with_bias and kt == KT - 1))
                        if with_bias:
                            nc.tensor.matmul(pv[:, :], ones_row[:, 0:128],
                                             brow_w(i, 0), start=False, stop=True)
                        nc.scalar.copy(
                            out=v_all[:, (j * CT + tt) * 512:(j * CT + tt) * 512 + 512],
                            in_=pv[:, :])

                    # delta state + prefix chain:  P[j+1] = P[j] + kseq^T [v|1]
                    skd = small.tile([128, KT], F32, tag="skd")
                    for kt in range(KT):
                        nc.vector.reduce_sum(
                            out=skd[:, kt:kt + 1],
                            in_=kT_all[:, kt, j * 256:(j + 1) * 256],
                            axis=mybir.AxisListType.X)
                        pd = psum.tile([128, 512], F32, tag="one", bufs=4)
                        for tt in range(CT):
                            ks = kseq[:, tt, kt * 128:(kt + 1) * 128]
                            nc.tensor.matmul(
                                pd[:, :], ks,
                                v_all[:, (j * CT + tt) * 512:(j * CT + tt) * 512 + 512],
                                start=(tt == 0), stop=(tt == CT - 1))
                        nc.vector.scalar_tensor_tensor(
                            out=Pb(j + 1, kt)[:, 0:E], in0=pd[:, :], scalar=1.0,
                            in1=Pb(j, kt)[:, 0:E], op0=ALU.mult, op1=ALU.add)
                    for kt in range(KT):
                        nc.vector.scalar_tensor_tensor(
                            out=Pb(j + 1, kt)[:, E:SKW], in0=skd[:, kt:kt + 1], scalar=1.0,
                            in1=Pb(j, kt)[:, E:SKW], op0=ALU.mult, op1=ALU.add)

            # ---------- boundary-state exchange (kick; masked-send AllGather) ----------
            contrib = work.tile([128, KT * SKW], BF16, tag="contrib", bufs=1)
            nc.vector.tensor_scalar_mul(contrib,
                                        P_all[:, NCH * KT * SKW:(NCH + 1) * KT * SKW],
                                        mcol[:, 1:2])
            cc_out = dram.tile([128, KT * SKW], BF16, tag="cc_out")
            cc_in = dram.tile([2 * 128, KT * SKW], BF16, tag="cc_in")
            nc.sync.dma_start(out=cc_out, in_=contrib)
            if cc:
                nc.gpsimd.collective_compute(
                    "AllGather", ALU.bypass, replica_groups=REPLICA_GROUPS,
                    ins=[cc_out.opt()], outs=[cc_in.opt()])
            else:
                nc.sync.dma_start(out=cc_in[0:128, :], in_=cc_out)
            nc.sync.dma_start(out=mR, in_=cc_in[0:128, :])
            nc.vector.tensor_scalar_mul(mR, mR, mcol[:, 0:1])

            # ---------- pass 2a (R-independent): q proj + scores + local den ----------
            qT2 = work.tile([128, 2 * KT, 512], BF16, tag="qT2", bufs=1)
            sm_all = work.tile([128, NCH * 384], BF16, tag="smA", bufs=1)
            pden = psum.tile([128, 2 * NCH], F32, tag="pden", bufs=1)
            for jp in range(0, NCH, 2):
                p2 = (jp // 2) * KT
                for fh in range(2):
                    pp = psum.tile([128, 1024], F32, tag="big2", bufs=1)
                    for fi in range(2):
                        ft = fh * 2 + fi
                        for kt in range(KT):
                            nc.tensor.matmul(
                                pp[:, fi * 512:(fi + 1) * 512],
                                wq[:, (0 * KT + kt) * 512 + ft * 128:(0 * KT + kt) * 512 + ft * 128 + 128],
                                hT_all[:, kt, jp * 256: jp * 256 + 512],
                                start=(kt == 0), stop=(kt == KT - 1))
                    et = work.tile([128, 1024], BF16, tag="phiE")
                    for fi in range(2):
                        ft = fh * 2 + fi
                        bc = bcol_w(i, 0, ft)
                        nc.scalar.activation(out=et[:, fi * 512:(fi + 1) * 512],
                                             in_=pp[:, fi * 512:(fi + 1) * 512],
                                             func=AF.Exp, bias=bc, scale=1.0)
                        nc.scalar.activation(out=qT2[:, p2 + ft, :],
                                             in_=pp[:, fi * 512:(fi + 1) * 512],
                                             func=AF.Relu, bias=bc, scale=1.0)
                    d = qT2[:, p2 + fh * 2: p2 + fh * 2 + 2, :]
                    nc.vector.scalar_tensor_tensor(out=d, in0=et, scalar=1.0,
                                                   in1=d, op0=ALU.min, op1=ALU.add)

                for j in (jp, jp + 1):
                    jo = (j & 1) * 256
                    # scoresT: cols 0:256 = t0 x (s0|s1); cols 256:384 = t1 x s1
                    ps = psum.tile([128, 384], F32, tag="sm", bufs=1)
                    for ft in range(KT):
                        nc.tensor.matmul(
                            ps[:, 0:256],
                            kT_all[:, ft, j * 256: j * 256 + 128],
                            qT2[:, p2 + ft, jo: jo + 256],
                            start=(ft == 0), stop=False)
                        nc.tensor.matmul(
                            ps[:, 256:384],
                            kT_all[:, ft, j * 256 + 128: j * 256 + 256],
                            qT2[:, p2 + ft, jo + 128: jo + 256],
                            start=False, stop=(ft == KT - 1))
                    sm = sm_all[:, j * 384:(j + 1) * 384]
                    nc.vector.tensor_mul(sm[:, 0:128], ps[:, 0:128], tril)
                    nc.scalar.copy(out=sm[:, 128:256], in_=ps[:, 128:256])
                    nc.vector.tensor_mul(sm[:, 256:384], ps[:, 256:384], tril)

                    # den (local part): column sums of masked scores.
                    # NOTE: start=True clears the whole PSUM bank, so only the
                    # very first matmul into pden may carry it.
                    dc = j * CT
                    nc.tensor.matmul(pden[:, dc:dc + 1], sm[:, 0:128], ones_col_bf,
                                     start=(j == 0), stop=False)
                    nc.tensor.matmul(pden[:, dc + 1:dc + 2], sm[:, 128:256], ones_col_bf,
                                     start=False, stop=False)
                    nc.tensor.matmul(pden[:, dc + 1:dc + 2], sm[:, 256:384], ones_col_bf,
                                     start=False, stop=False)

            # ---------- pass 2b (needs R): num/den finalize + o-proj + LN2 ----------
            mv2 = small.tile([128, 2 * NT8], F32, tag="mv2")
            for jp in range(0, NCH, 2):
                p2 = (jp // 2) * KT
                for j in (jp, jp + 1):
                    jo = (j & 1) * 256
                    if j == 0:
                        Peff = mR  # exclusive prefix of chunk 0 is zero
                    else:
                        Peff = work.tile([128, KT * SKW], BF16, tag="Peff")
                        nc.vector.scalar_tensor_tensor(
                            out=Peff, in0=P_all[:, j * KT * SKW:(j + 1) * KT * SKW],
                            scalar=1.0, in1=mR, op0=ALU.mult, op1=ALU.add)

                    sm = sm_all[:, j * 384:(j + 1) * 384]
                    # num, seq-major: [s, e'] = masked-scores @ v + q @ KV.
                    pn0 = psum.tile([128, 512], F32, tag="one", bufs=4)
                    pn1 = psum.tile([128, 512], F32, tag="one", bufs=4)
                    v0 = v_all[:, (j * CT + 0) * 512:(j * CT + 0) * 512 + 512]
                    v1 = v_all[:, (j * CT + 1) * 512:(j * CT + 1) * 512 + 512]
                    nc.tensor.matmul(pn0[:, :], sm[:, 0:128], v0,
                                     start=True, stop=False)
                    nc.tensor.matmul(pn1[:, :], sm[:, 128:256], v0,
                                     start=True, stop=False)
                    nc.tensor.matmul(pn1[:, :], sm[:, 256:384], v1,
                                     start=False, stop=False)
                    for kt in range(KT):
                        nc.tensor.matmul(pn0[:, :],
                                         qT2[:, p2 + kt, jo: jo + 128],
                                         Peff[:, kt * SKW: kt * SKW + E],
                                         start=False, stop=(kt == KT - 1))
                        nc.tensor.matmul(pn1[:, :],
                                         qT2[:, p2 + kt, jo + 128: jo + 256],
                                         Peff[:, kt * SKW: kt * SKW + E],
                                         start=False, stop=(kt == KT - 1))

                    # den finalize: + q . s_k
                    dc = j * CT
                    for st in range(CT):
                        for kt in range(KT):
                            nc.tensor.matmul(
                                pden[:, dc + st:dc + st + 1],
                                qT2[:, p2 + kt, jo + st * 128: jo + st * 128 + 128],
                                Peff[:, kt * SKW + E: kt * SKW + SKW],
                                start=False, stop=(st == CT - 1 and kt == KT - 1))
                    den = small.tile([128, CT], F32, tag="den")
                    nc.scalar.activation(out=den, in_=pden[:, dc:dc + CT],
                                         func=AF.Identity,
                                         bias=eps_den, scale=1.0)
                    rden = small.tile([128, CT], F32, tag="rden")
                    nc.vector.reciprocal(out=rden, in_=den)

                    # attn = num/den (seq-major, natural per-partition scale),
                    # then DMA-transpose to feature-major for the o-projection
                    attn = work.tile([128, CT * 512], BF16, tag="numT")
                    nc.scalar.activation(out=attn[:, 0:512], in_=pn0[:, :],
                                         func=AF.Copy, scale=rden[:, 0:1])
                    nc.scalar.activation(out=attn[:, 512:1024], in_=pn1[:, :],
                                         func=AF.Copy, scale=rden[:, 1:2])
                    attnT = work.tile([128, KT, 256], BF16, tag="attnT")
                    for st in range(CT):
                        nc.sync.dma_start_transpose(
                            out=attnT[:, :, st * 128:(st + 1) * 128],
                            in_=attn[:, st * 512:(st + 1) * 512])

                    # o-projection + residual
                    for st in range(CT):
                        po = psum.tile([128, 512], F32, tag="one", bufs=4)
                        for mt in range(KT):
                            nc.tensor.matmul(po[:, :],
                                             attnT[:, mt, st * 128:(st + 1) * 128],
                                             wo[:, (0 * KT + mt) * 512:(0 * KT + mt) * 512 + 512],
                                             start=(mt == 0),
                                             stop=(not with_bias and mt == KT - 1))
                        if with_bias:
                            nc.tensor.matmul(po[:, :], ones_row[:, 0:128],
                                             brow_w(i, 1), start=False, stop=True)
                        ls = lat[:, (j * CT + st) * 512:(j * CT + st) * 512 + 512]
                        nc.vector.scalar_tensor_tensor(out=ls, in0=po[:, :],
                                                       scalar=1.0,
                                                       in1=ls, op0=ALU.mult, op1=ALU.add)

                    # LN2 stats here (post-residual); scales batched per pair
                    for ct in range(CT):
                        ln_stats(mv2, j * CT + ct, j * CT + ct)

                # LN2 normalize + transpose sweep for this pair (DVE + DMA)
                negm2, rstd2 = ln_scales(mv2[:, jp * CT * 2: jp * CT * 2 + 8],
                                         4, "l2")
                for c4 in range(4):
                    ct8 = jp * CT + c4
                    y2 = work.tile([128, 512], BF16, tag="y2", bufs=2)
                    ln_norm(y2, ct8, negm2, rstd2, c4)
                    nc.sync.dma_start_transpose(
                        out=h2T_all[:, :, ct8 * 128:(ct8 + 1) * 128],
                        in_=y2)

            # ---------- pass 3: FFN (+ next layer's LN1, pipelined) ----------
            mv1n = small.tile([128, 2 * NT8], F32, tag="mv1")
            for jp in range(0, NCH, 2):
                h1T = work.tile([128, KT, 512], BF16, tag="h1T", bufs=1)
                for fh in range(2):
                    ph1 = psum.tile([128, 1024], F32, tag="big2", bufs=1)
                    for fi in range(2):
                        ft = fh * 2 + fi
                        for kt in range(KT):
                            nc.tensor.matmul(
                                ph1[:, fi * 512:(fi + 1) * 512],
                                wo[:, (1 * KT + kt) * 512 + ft * 128:(1 * KT + kt) * 512 + ft * 128 + 128],
                                h2T_all[:, kt, jp * 256: jp * 256 + 512],
                                start=(kt == 0), stop=(kt == KT - 1))
                    for fi in range(2):
                        ft = fh * 2 + fi
                        nc.scalar.activation(out=h1T[:, ft, :],
                                             in_=ph1[:, fi * 512:(fi + 1) * 512],
                                             func=AF.Gelu, bias=bcol_w(i, 2, ft),
                                             scale=1.0)

                for j in (jp, jp + 1):
                    jo = (j & 1) * 256
                    for st in range(CT):
                        pf = psum.tile([128, 512], F32, tag="one", bufs=4)
                        for mt in range(KT):
                            nc.tensor.matmul(
                                pf[:, :],
                                h1T[:, mt, jo + st * 128: jo + st * 128 + 128],
                                wo[:, (2 * KT + mt) * 512:(2 * KT + mt) * 512 + 512],
                                start=(mt == 0),
                                stop=(not with_bias and mt == KT - 1))
                        if with_bias:
                            nc.tensor.matmul(pf[:, :], ones_row[:, 0:128],
                                             brow_w(i, 2), start=False, stop=True)
                        ls = lat[:, (j * CT + st) * 512:(j * CT + st) * 512 + 512]
                        nc.vector.scalar_tensor_tensor(out=ls, in0=pf[:, :], scalar=1.0,
                                                       in1=ls, op0=ALU.mult, op1=ALU.add)
                        if i_rep < L * repeat - 1:
                            ln_stats(mv1n, j * CT + st, j * CT + st)
                if i_rep < L * repeat - 1:
                    ln1_sweep(mv1n, jp)

        # =========== output projection ===========
        with tc.tile_pool(name="tail", bufs=1) as tailp:
            latT = tailp.tile([128, KT, SH], BF16)
            for ct8 in range(NT8):
                latb = work.tile([128, 512], BF16, tag="y2")
                nc.vector.tensor_copy(out=latb, in_=lat[:, ct8 * 512:(ct8 + 1) * 512])
                nc.sync.dma_start_transpose(
                    out=latT[:, :, ct8 * 128:(ct8 + 1) * 128], in_=latb)

            out_sb = tailp.tile([COUT, SH], F32)
            for sb in range(SH // 512):
                pout = psum.tile([COUT, 512], F32, tag="one", bufs=4)
                for kt in range(KT):
                    nc.tensor.matmul(pout[:, :],
                                     outWT[:, kt * COUT:(kt + 1) * COUT],
                                     latT[:, kt, sb * 512:(sb + 1) * 512],
                                     start=(kt == 0),
                                     stop=(not with_bias and kt == KT - 1))
                if with_bias:
                    nc.tensor.matmul(pout[:, :], outb_row, ones_row,
                                     start=False, stop=True)
                nc.scalar.copy(out=out_sb[:, sb * 512:(sb + 1) * 512], in_=pout[:, :])
            dma(out=out_d[:, :], in_=out_sb)


# ---------------- host side ----------------

_CACHE = threading.local()


def _get_program(with_bias=False):
    key = f"nc_{with_bias}"
    if not hasattr(_CACHE, key):
        setattr(_CACHE, key, build_program(with_bias=with_bias))
    return getattr(_CACHE, key)


def _needs_bias(inputs):
    f32 = np.float32
    ln1_b = np.asarray(inputs["ln1_b"], f32)
    ln2_b = np.asarray(inputs["ln2_b"], f32)
    vals = [np.asarray(inputs[k], f32) for k in
            ("in_b", "cc_b", "out_b", "bo", "c2_b")]
    bv_eff = np.asarray(inputs["bv"], f32) + np.einsum(
        "loe,le->lo", np.asarray(inputs["Wv"], f32), ln1_b)
    vals.append(bv_eff)
    return any(np.abs(v).max() > 0 for v in vals)


def _prep_shared(inputs):
    f32 = np.float32
    inW = np.asarray(inputs["in_W"], f32)      # [E, CIN]
    in_b = np.asarray(inputs["in_b"], f32)
    ccW = np.asarray(inputs["cc_W"], f32)      # [E, E, 3]
    cc_b = np.asarray(inputs["cc_b"], f32)
    outW = np.asarray(inputs["out_W"], f32)    # [COUT, E]
    out_b = np.asarray(inputs["out_b"], f32)

    ccWT = np.zeros((128, 3 * KT * 512), f32)
    for d in range(3):
        WT = ccW[:, :, d].T  # [e_in, e_out]
        for kt in range(KT):
            ccWT[:, (d * KT + kt) * 512:(d * KT + kt) * 512 + 512] = \
                WT[kt * 128:(kt + 1) * 128, :]

    ln1_g = np.asarray(inputs["ln1_g"], f32); ln1_b = np.asarray(inputs["ln1_b"], f32)
    ln2_g = np.asarray(inputs["ln2_g"], f32); ln2_b = np.asarray(inputs["ln2_b"], f32)

    wpack = np.zeros((128, L * 6 * KT * 512), f32)
    brow = np.zeros((1, BROW_N), f32)
    bcol = np.zeros((128, L * 3 * KT), f32)
    for i in range(L):
        biases = {}
        for w, (Wn, bn, g, bb) in enumerate((
                ("Wq", "bq", ln1_g[i], ln1_b[i]),
                ("Wk", "bk", ln1_g[i], ln1_b[i]),
                ("Wv", "bv", ln1_g[i], ln1_b[i]),
                ("Wo", "bo", None, None),
                ("c1_W", "c1_b", ln2_g[i], ln2_b[i]),
                ("c2_W", "c2_b", None, None))):
            W = np.asarray(inputs[Wn], f32)[i]          # [E_out, E_in]
            bias = np.asarray(inputs[bn], f32)[i].copy()
            if g is not None:
                WT = (W * g[None, :]).T                  # fold LN gain
                bias = bias + W @ bb                     # fold LN bias
            else:
                WT = W.T
            for kt in range(KT):
                wpack[:, (i * 6 + w) * KT * 512 + kt * 512:
                      (i * 6 + w) * KT * 512 + kt * 512 + 512] = \
                    WT[kt * 128:(kt + 1) * 128, :]
            biases[w] = bias
        # rows: bv, bo, c2b
        brow[0, (i * 3 + 0) * 512:(i * 3 + 0) * 512 + 512] = biases[2]
        brow[0, (i * 3 + 1) * 512:(i * 3 + 1) * 512 + 512] = biases[3]
        brow[0, (i * 3 + 2) * 512:(i * 3 + 2) * 512 + 512] = biases[5]
        # cols: bq, bk, c1b
        for which, w in ((0, 0), (1, 1), (2, 4)):
            for kt in range(KT):
                bcol[:, (i * 3 + which) * KT + kt] = biases[w][kt * 128:(kt + 1) * 128]

    inWT = inW.T  # [CIN, E]
    outWT = np.zeros((128, KT * COUT), f32)
    for kt in range(KT):
        outWT[:, kt * COUT:(kt + 1) * COUT] = outW.T[kt * 128:(kt + 1) * 128, :]

    brow[0, L * 3 * 512: L * 3 * 512 + 512] = in_b
    brow[0, L * 3 * 512 + 512: L * 3 * 512 + 1024] = cc_b
    brow[0, L * 3 * 512 + 1024: L * 3 * 512 + 1024 + COUT] = out_b

    tril = np.tril(np.ones((128, 128), f32)).T  # keep t<=s in [t,s] layout

    return {
        "inWT": inWT.astype(BF),
        "ccWT": ccWT.astype(BF),
        "wpack": wpack.astype(BF),
        "outWT": outWT.astype(BF),
        "brow": brow.astype(BF),
        "bcol": bcol,
        "tril": tril.astype(BF),
        "ones_row": np.ones((1, 512), f32).astype(BF),
        "ones_col_bf": np.ones((128, 1), f32).astype(BF),
    }


def _prep_core_inputs(shared, inputs, b, h):
    f32 = np.float32
    x = np.asarray(inputs["x"], f32)
    s0 = h * SH
    x_sl = np.zeros((CIN, SH + 2), f32)
    lo = max(0, s0 - 2)
    x_sl[:, 2 - (s0 - lo):] = x[b, :, lo:s0 + SH]
    halo = np.ones((1, SH + 2), f32)
    if h == 0:
        halo[0, :2] = 0.0
    mcol = np.zeros((128, 2), f32)
    mcol[:, 0] = float(h)
    mcol[:, 1] = 1.0 - float(h)
    m = dict(shared)
    m["x_sl"] = x_sl.astype(BF)
    m["halo"] = halo.astype(BF)
    m["mcol"] = mcol
    return m


def _run(inputs, **kw):
    nc = _get_program(with_bias=_needs_bias(inputs))
    shared = _prep_shared(inputs)
    in_maps = []
    for core in range(NCORES):
        b, h = core // 2, core % 2
        in_maps.append(_prep_core_inputs(shared, inputs, b, h))
    return run_bass_kernel_spmd(nc, in_maps, core_ids=list(range(NCORES)), **kw)


def kernel(**inputs):
    res = _run(inputs)
    out = np.zeros((B, COUT, S), np.float32)
    for core in range(NCORES):
        b, h = core // 2, core % 2
        out[b, :, h * SH:(h + 1) * SH] = res.results[core]["out"]
    return out


def bench(inputs, trace_cores=(0, 1), tmpdir=None):
    """Run with NTFF tracing; returns BassKernelResults with exec_time_ns."""
    return _run(inputs, trace=True, trace_cores=list(trace_cores), tmpdir=tmpdir)



# revision 29
# speedup vs baseline: 135.8536x; 1.0605x over previous
"""Recurrent linear-attention transformer on 8 Trainium2 NeuronCores.

Sharding: 4-way data parallel over batch x 2-way sequence split (halves of
S=2048). Causal linear attention is computed in chunked form; the only
cross-core dependency is the cumulative (k^T v, sum k) state at the half
boundary, exchanged once per layer via a pairwise AllReduce.

Per-core layout conventions (SBUF 2-D tensors, 128 partitions):
  lat   f32  [128, 8*512]    seq-major residual: c-tile ct8 block, cols = e
  hT    bf16 [128, 4*4*256]  feature-major LN1 output: (chunk, kt) blocks
  kT    bf16 [128, 4*4*256]  phi(k) feature-major: (chunk, ft) blocks, cols=t
  v     bf16 [128, 4*2*512]  v seq-major: (chunk, tt) blocks, cols = e'
  P_all bf16 [128, 5*4*513]  prefix states: (j, kt) -> [KV[e,e'] | s_k[e]]
All matmul operands bf16, PSUM/stats/residual f32.
"""

import threading

import numpy as np
import ml_dtypes

import concourse.bass as bass
import concourse.bacc as bacc
import concourse.tile as tile
import concourse.mybir as mybir
from concourse.bass_utils import run_bass_kernel_spmd

AF = mybir.ActivationFunctionType
ALU = mybir.AluOpType
F32 = mybir.dt.float32
BF16 = mybir.dt.bfloat16
BF = ml_dtypes.bfloat16

L, B, CIN, COUT, E, S = 4, 4, 64, 64, 512, 2048
NCORES = 8
SH = S // 2          # per-core sequence half
C = 256              # attention chunk
NCH = SH // C        # 4 chunks
CT = C // 128        # 2 c-tiles per chunk
KT = E // 128        # 4 feature tiles
NT8 = SH // 128      # 8 seq tiles per half
EPS = 1e-6
LN_EPS = 1e-5
SKW = E + 1          # 513: KV block plus s_k column

REPLICA_GROUPS = [[0, 1], [2, 3], [4, 5], [6, 7]]

# brow layout (bf16): per-layer [bv, bo, c2b] rows, then in_b, cc_b, out_b
BROW_N = L * 3 * 512 + 512 + 512 + COUT


def build_program(cc=True, repeat=1, with_bias=False, body_repeat=1):
    nc = bacc.Bacc("TRN2", target_bir_lowering=False, debug=False,
                   num_devices=NCORES)

    x_d = nc.dram_tensor("x_sl", [CIN, SH + 2], BF16, kind="ExternalInput")
    inWT_d = nc.dram_tensor("inWT", [CIN, E], BF16, kind="ExternalInput")
    ccWT_d = nc.dram_tensor("ccWT", [128, 3 * KT * 512], BF16, kind="ExternalInput")
    wpack_d = nc.dram_tensor("wpack", [128, L * 6 * KT * 512], BF16, kind="ExternalInput")
    outWT_d = nc.dram_tensor("outWT", [128, KT * COUT], BF16, kind="ExternalInput")
    brow_d = nc.dram_tensor("brow", [1, BROW_N], BF16, kind="ExternalInput")
    bcol_d = nc.dram_tensor("bcol", [128, L * 3 * KT], F32, kind="ExternalInput")
    tril_d = nc.dram_tensor("tril", [128, 128], BF16, kind="ExternalInput")
    mcol_d = nc.dram_tensor("mcol", [128, 2], F32, kind="ExternalInput")  # [m, 1-m]
    halo_d = nc.dram_tensor("halo", [1, SH + 2], BF16, kind="ExternalInput")
    ones_row_d = nc.dram_tensor("ones_row", [1, 512], BF16, kind="ExternalInput")
    ones_col_bf_d = nc.dram_tensor("ones_col_bf", [128, 1], BF16, kind="ExternalInput")

    out_d = nc.dram_tensor("out", [COUT, SH], F32, kind="ExternalOutput")

    with tile.TileContext(nc, num_cores=NCORES) as tc:
        with tc.tile_pool(name="singles", bufs=1) as singles:
            dma = nc.sync.dma_start
            cst = {}
            cst["outWT"] = singles.tile([128, KT * COUT], BF16)
            dma(out=cst["outWT"], in_=outWT_d[:, :])
            cst["brow"] = singles.tile([1, BROW_N], BF16)
            dma(out=cst["brow"], in_=brow_d[:, :])
            cst["bcol"] = singles.tile([128, L * 3 * KT], F32)
            dma(out=cst["bcol"], in_=bcol_d[:, :])
            cst["tril"] = singles.tile([128, 128], BF16)
            dma(out=cst["tril"], in_=tril_d[:, :])
            cst["mcol"] = singles.tile([128, 2], F32)
            dma(out=cst["mcol"], in_=mcol_d[:, :])
            cst["halo"] = singles.tile([1, SH + 2], BF16)
            dma(out=cst["halo"], in_=halo_d[:, :])
            cst["ones_row"] = singles.tile([1, 512], BF16)
            dma(out=cst["ones_row"], in_=ones_row_d[:, :])
            cst["ones_col_bf"] = singles.tile([128, 1], BF16)
            dma(out=cst["ones_col_bf"], in_=ones_col_bf_d[:, :])
            cst["inWT"] = singles.tile([CIN, E], BF16)
            dma(out=cst["inWT"], in_=inWT_d[:, :])
            cst["ccWT"] = singles.tile([128, 3 * KT * 512], BF16)
            dma(out=cst["ccWT"], in_=ccWT_d[:, :])
            cst["eps_den"] = singles.tile([128, 1], F32)
            nc.vector.memset(cst["eps_den"], EPS)
            cst["magic_i"] = singles.tile([128, NT8], mybir.dt.int32)
            nc.vector.memset(cst["magic_i"], 0x5F3759DF)
            for _body in range(body_repeat):
                _emit(nc, tc, cst, x_d, wpack_d, out_d, cc=cc, repeat=repeat,
                      with_bias=with_bias)
    nc.compile()
    return nc


def _emit(nc, tc, cst, x_d, wpack_d, out_d, cc=True, repeat=1,
          with_bias=False):
    import contextlib
    ctx = contextlib.ExitStack()
    with ctx:
        persist = ctx.enter_context(tc.tile_pool(name="persist", bufs=1))
        small = ctx.enter_context(tc.tile_pool(name="small", bufs=4))
        psum = ctx.enter_context(tc.tile_pool(name="psum", bufs=2, space="PSUM"))
        dram = ctx.enter_context(tc.tile_pool(name="dram", bufs=2, space="DRAM"))

        dma = nc.sync.dma_start

        # ---- constants (resident; loaded once in build_program) ----
        outWT = cst["outWT"]
        brow = cst["brow"]
        bcol = cst["bcol"]
        tril = cst["tril"]
        mcol = cst["mcol"]
        halo = cst["halo"]
        ones_row = cst["ones_row"]
        ones_col_bf = cst["ones_col_bf"]
        inWT_c = cst["inWT"]
        ccWT_c = cst["ccWT"]
        eps_den = cst["eps_den"]
        magic_i = cst["magic_i"]

        inb_row = brow[:, L * 3 * 512: L * 3 * 512 + 512]
        ccb_row = brow[:, L * 3 * 512 + 512: L * 3 * 512 + 1024]
        outb_row = brow[:, L * 3 * 512 + 1024: L * 3 * 512 + 1024 + COUT]

        def brow_w(i, w):
            # w: 0=bv 1=bo 2=c2b
            return brow[:, (i * 3 + w) * 512:(i * 3 + w) * 512 + 512]

        def bcol_w(i, which, kt):
            # which: 0=bq 1=bk 2=c1b
            c = (i * 3 + which) * KT + kt
            return bcol[:, c:c + 1]

        # ---- persistent state ----
        lat = persist.tile([128, NT8 * 512], F32)
        P_all = persist.tile([128, (NCH + 1) * KT * SKW], BF16)
        nc.vector.memset(P_all[:, 0:KT * SKW], 0.0)
        # feature-tile-major activations: [within-tile row, tile, seq col]
        hT_all = persist.tile([128, KT, SH // 4 * 4], BF16)   # [e%128, kt, s]
        kT_all = persist.tile([128, KT, SH // 4 * 4], BF16)   # [feat%128, ft, t]
        h2T_all = persist.tile([128, KT, SH // 4 * 4], BF16)
        v_all = persist.tile([128, NCH * CT * 512], BF16)
        mR = persist.tile([128, KT * SKW], BF16)

        def Pb(j, kt):
            o = (j * KT + kt) * SKW
            return P_all[:, o:o + SKW]

        # =========== input projection + causal conv (scoped pool) ===========
        with tc.tile_pool(name="convp", bufs=1) as convp:
            x_sb = convp.tile([CIN, SH + 2], BF16)
            dma(out=x_sb, in_=x_d[:, :])
            inWT = inWT_c
            ccWT = ccWT_c
            z = convp.tile([128, KT * (SH + 2)], BF16)
            for kt in range(KT):
                for s0, ns in ((0, 512), (512, 512), (1024, 2)):
                    pz = psum.tile([128, 512], F32, tag="one", bufs=4)
                    nc.tensor.matmul(pz[:, :ns],
                                     inWT[:, kt * 128:(kt + 1) * 128],
                                     x_sb[:, s0:s0 + ns], start=True,
                                     stop=not with_bias)
                    if with_bias:
                        nc.tensor.matmul(pz[:, :ns],
                                         inb_row[:, kt * 128:(kt + 1) * 128],
                                         halo[:, s0:s0 + ns], start=False, stop=True)
                    nc.scalar.copy(out=z[:, kt * (SH + 2) + s0: kt * (SH + 2) + s0 + ns],
                                   in_=pz[:, :ns])

            for ct8 in range(NT8):
                pc = psum.tile([128, 512], F32, tag="one", bufs=4)
                first = True
                for d in range(3):
                    for kt in range(KT):
                        zo = kt * (SH + 2) + ct8 * 128 + d
                        nc.tensor.matmul(pc[:, :],
                                         z[:, zo:zo + 128],
                                         ccWT[:, (d * KT + kt) * 512:(d * KT + kt) * 512 + 512],
                                         start=first,
                                         stop=(not with_bias and d == 2 and kt == KT - 1))
                        first = False
                if with_bias:
                    nc.tensor.matmul(pc[:, :], ones_row[:, 0:128], ccb_row,
                                     start=False, stop=True)
                nc.scalar.copy(out=lat[:, ct8 * 512:(ct8 + 1) * 512], in_=pc[:, :])

        # layer-loop pools (opened after conv pool closes)
        wq_pool = ctx.enter_context(tc.tile_pool(name="wq_pool", bufs=2))
        wo_pool = ctx.enter_context(tc.tile_pool(name="wo_pool", bufs=2))
        work = ctx.enter_context(tc.tile_pool(name="work", bufs=2))

        # =========== transformer layers ===========
        def ln_stats(mv8, ct8, sl):
            """bn stats of lat c-tile ct8 into mv8[:, 2*sl:2*sl+2]."""
            stats = small.tile([128, 6], F32, tag="lnst")
            nc.vector.bn_stats(out=stats, in_=lat[:, ct8 * 512:(ct8 + 1) * 512])
            nc.vector.bn_aggr(out=mv8[:, 2 * sl:2 * sl + 2], in_=stats)

        def ln_scales(mv8, n, tag):
            """From interleaved [mean,var] pairs build nmr=-mean*rstd [128,n]
            and rstd [128,n] (Newton rsqrt on DVE; no ACT table involved)."""
            negm = small.tile([128, n], F32, tag=tag + "nm")
            var = small.tile([128, n], F32, tag=tag + "va")
            rstd = small.tile([128, n], F32, tag=tag + "rs")
            tmp = small.tile([128, n], F32, tag=tag + "tm")
            nmr = small.tile([128, n], F32, tag=tag + "nmr")
            mvv = mv8[:, 0:2 * n].rearrange("p (n two) -> p n two", two=2)
            nc.vector.tensor_scalar_mul(negm, mvv[:, :, 0], -1.0)
            nc.vector.tensor_scalar_add(var, mvv[:, :, 1], LN_EPS)
            vi = var.bitcast(mybir.dt.int32)
            ri = rstd.bitcast(mybir.dt.int32)
            nc.vector.tensor_scalar(ri, vi, 1, None, op0=ALU.arith_shift_right)
            nc.vector.tensor_tensor(ri, magic_i[:, 0:n], ri, op=ALU.subtract)
            for _ in range(2):
                nc.vector.tensor_mul(tmp, rstd, rstd)
                nc.vector.tensor_mul(tmp, tmp, var)
                nc.vector.tensor_scalar(tmp, tmp, -0.5, 1.5,
                                        op0=ALU.mult, op1=ALU.add)
                nc.vector.tensor_mul(rstd, rstd, tmp)
            nc.vector.tensor_mul(nmr, negm, rstd)
            return nmr, rstd

        def ln_norm(dst_y, ct8, nmr, rstd, sl):
            # (x - m) * r as r*x + (-m*r) on DVE (ptr scalars)
            nc.vector.tensor_scalar(dst_y, lat[:, ct8 * 512:(ct8 + 1) * 512],
                                    rstd[:, sl:sl + 1], nmr[:, sl:sl + 1],
                                    op0=ALU.mult, op1=ALU.add)

        def ln1_sweep(mv, jp):
            """Per-pair LN1 finish: rsqrt batch, normalize, transpose to hT."""
            negm, rstd = ln_scales(mv[:, jp * CT * 2: jp * CT * 2 + 8], 4, "l1")
            y = work.tile([128, 4 * 512], BF16, tag="y", bufs=2)
            for c4 in range(4):
                ln_norm(y[:, c4 * 512:(c4 + 1) * 512], jp * CT + c4,
                        negm, rstd, c4)
            for c4 in range(4):
                nc.sync.dma_start_transpose(
                    out=hT_all[:, :, jp * 256 + c4 * 128: jp * 256 + c4 * 128 + 128],
                    in_=y[:, c4 * 512:(c4 + 1) * 512])

        def pair_proj_phi(wt, w, jp, dst3, i, which):
            """Feature-major projection for a chunk pair with phi applied.
            dst3: 3D [128, KT, SH] tile written at cols [jp*256, jp*256+512)."""
            for fh in range(2):
                pp = psum.tile([128, 1024], F32, tag="big2", bufs=1)
                for fi in range(2):
                    ft = fh * 2 + fi
                    for kt in range(KT):
                        nc.tensor.matmul(
                            pp[:, fi * 512:(fi + 1) * 512],
                            wt[:, (w * KT + kt) * 512 + ft * 128:(w * KT + kt) * 512 + ft * 128 + 128],
                            hT_all[:, kt, jp * 256: jp * 256 + 512],
                            start=(kt == 0), stop=(kt == KT - 1))
                et = work.tile([128, 1024], BF16, tag="phiE")
                for fi in range(2):
                    ft = fh * 2 + fi
                    bc = bcol_w(i, which, ft)
                    nc.scalar.activation(out=et[:, fi * 512:(fi + 1) * 512],
                                         in_=pp[:, fi * 512:(fi + 1) * 512],
                                         func=AF.Exp, bias=bc, scale=1.0)
                    nc.scalar.activation(out=dst3[:, ft, jp * 256: jp * 256 + 512],
                                         in_=pp[:, fi * 512:(fi + 1) * 512],
                                         func=AF.Relu, bias=bc, scale=1.0)
                d = dst3[:, fh * 2: fh * 2 + 2, jp * 256: jp * 256 + 512]
                nc.vector.scalar_tensor_tensor(out=d, in0=et, scalar=1.0, in1=d,
                                               op0=ALU.min, op1=ALU.add)

        # layer-0 LN1 (later layers pipeline theirs into pass 3)
        mv1 = small.tile([128, 2 * NT8], F32, tag="mv1")
        for ct8 in range(NT8):
            ln_stats(mv1, ct8, ct8)
        for jp in range(0, NCH, 2):
            ln1_sweep(mv1, jp)

        def load_layer_weights(i):
            wq = wq_pool.tile([128, 3 * KT * 512], BF16, tag="wq", name="wq")
            wo = wo_pool.tile([128, 3 * KT * 512], BF16, tag="wo", name="wo")
            for wti in range(3):
                dma(out=wq[:, wti * KT * 512:(wti + 1) * KT * 512],
                    in_=wpack_d[:, (i * 6 + wti) * KT * 512:(i * 6 + wti + 1) * KT * 512])
                dma(out=wo[:, wti * KT * 512:(wti + 1) * KT * 512],
                    in_=wpack_d[:, (i * 6 + 3 + wti) * KT * 512:(i * 6 + 4 + wti) * KT * 512])
            return wq, wo

        wnext = load_layer_weights(0)
        for i_rep in range(L * repeat):
            i = i_rep % L
            wq, wo = wnext

            # ---------- pass 1: k, v, local chunk states ----------
            for jp in range(0, NCH, 2):
                pair_proj_phi(wq, 1, jp, kT_all, i, 1)

                for j in (jp, jp + 1):
                    # kseq[t%128, tt, e] = phi(k)[e, t] transposed
                    kseq = work.tile([128, CT, 512], BF16, tag="kseq", bufs=2)
                    for ft in range(KT):
                        nc.sync.dma_start_transpose(
                            out=kseq[:, :, ft * 128:(ft + 1) * 128],
                            in_=kT_all[:, ft, j * 256:(j + 1) * 256])

                    # v projection (seq-major)
                    for tt in range(CT):
                        pv = psum.tile([128, 512], F32, tag="one", bufs=4)
                        for kt in range(KT):
                            nc.tensor.matmul(
                                pv[:, :],
                                hT_all[:, kt, j * 256 + tt * 128: j * 256 + tt * 128 + 128],
                                wq[:, (2 * KT + kt) * 512:(2 * KT + kt) * 512 + 512],
                                start=(kt == 0),
                                stop=(not with_bias and kt == KT - 1))
                        if with_bias:
                            nc.tensor.matmul(pv[:, :], ones_row[:, 0:128],
                                             brow_w(i, 0), start=False, stop=True)
                        nc.scalar.copy(
                            out=v_all[:, (j * CT + tt) * 512:(j * CT + tt) * 512 + 512],
                            in_=pv[:, :])

                    # delta state + prefix chain:  P[j+1] = P[j] + kseq^T [v|1]
                    # s_k delta via tiny PE matmuls (kseq^T @ ones) into PSUM
                    skp = psum.tile([128, KT], F32, tag="sm", bufs=1)
                    for kt in range(KT):
                        for tt in range(CT):
                            nc.tensor.matmul(
                                skp[:, kt:kt + 1],
                                kseq[:, tt, kt * 128:(kt + 1) * 128],
                                ones_col_bf,
                                start=(kt == 0 and tt == 0), stop=(kt == KT - 1 and tt == CT - 1))
                    for kt in range(KT):
                        pd = psum.tile([128, 512], F32, tag="one", bufs=4)
                        for tt in range(CT):
                            ks = kseq[:, tt, kt * 128:(kt + 1) * 128]
                            nc.tensor.matmul(
                                pd[:, :], ks,
                                v_all[:, (j * CT + tt) * 512:(j * CT + tt) * 512 + 512],
                                start=(tt == 0), stop=(tt == CT - 1))
                        nc.vector.scalar_tensor_tensor(
                            out=Pb(j + 1, kt)[:, 0:E], in0=pd[:, :], scalar=1.0,
                            in1=Pb(j, kt)[:, 0:E], op0=ALU.mult, op1=ALU.add)
                    for kt in range(KT):
                        nc.vector.scalar_tensor_tensor(
                            out=Pb(j + 1, kt)[:, E:SKW], in0=skp[:, kt:kt + 1], scalar=1.0,
                            in1=Pb(j, kt)[:, E:SKW], op0=ALU.mult, op1=ALU.add)

            # ---------- boundary-state exchange (kick; masked-send AllGather) ----------
            contrib = work.tile([128, KT * SKW], BF16, tag="contrib", bufs=1)
            nc.vector.tensor_scalar_mul(contrib,
                                        P_all[:, NCH * KT * SKW:(NCH + 1) * KT * SKW],
                                        mcol[:, 1:2])
            cc_out = dram.tile([128, KT * SKW], BF16, tag="cc_out")
            cc_in = dram.tile([2 * 128, KT * SKW], BF16, tag="cc_in")
            nc.sync.dma_start(out=cc_out, in_=contrib)
            if cc:
                nc.gpsimd.collective_compute(
                    "AllGather", ALU.bypass, replica_groups=REPLICA_GROUPS,
                    ins=[cc_out.opt()], outs=[cc_in.opt()])
            else:
                nc.sync.dma_start(out=cc_in[0:128, :], in_=cc_out)
            nc.sync.dma_start(out=mR, in_=cc_in[0:128, :])
            nc.vector.tensor_scalar_mul(mR, mR, mcol[:, 0:1])

            # prefetch next layer's weights into the exchange window
            if i_rep < L * repeat - 1:
                wnext = load_layer_weights((i_rep + 1) % L)

            # ---------- pass 2a (R-independent): q proj + scores + local den ----------
            qT2 = work.tile([128, 2 * KT, 512], BF16, tag="qT2", bufs=1)
            sm_all = work.tile([128, NCH * 384], BF16, tag="smA", bufs=1)
            pden = psum.tile([128, 2 * NCH], F32, tag="pden", bufs=1)
            pn_pre = {}
            for jp in range(0, NCH, 2):
                p2 = (jp // 2) * KT
                for fh in range(2):
                    pp = psum.tile([128, 1024], F32, tag="big2", bufs=1)
                    for fi in range(2):
                        ft = fh * 2 + fi
                        for kt in range(KT):
                            nc.tensor.matmul(
                                pp[:, fi * 512:(fi + 1) * 512],
                                wq[:, (0 * KT + kt) * 512 + ft * 128:(0 * KT + kt) * 512 + ft * 128 + 128],
                                hT_all[:, kt, jp * 256: jp * 256 + 512],
                                start=(kt == 0), stop=(kt == KT - 1))
                    et = work.tile([128, 1024], BF16, tag="phiE")
                    for fi in range(2):
                        ft = fh * 2 + fi
                        bc = bcol_w(i, 0, ft)
                        nc.scalar.activation(out=et[:, fi * 512:(fi + 1) * 512],
                                             in_=pp[:, fi * 512:(fi + 1) * 512],
                                             func=AF.Exp, bias=bc, scale=1.0)
                        nc.scalar.activation(out=qT2[:, p2 + ft, :],
                                             in_=pp[:, fi * 512:(fi + 1) * 512],
                                             func=AF.Relu, bias=bc, scale=1.0)
                    d = qT2[:, p2 + fh * 2: p2 + fh * 2 + 2, :]
                    nc.vector.scalar_tensor_tensor(out=d, in0=et, scalar=1.0,
                                                   in1=d, op0=ALU.min, op1=ALU.add)

                for j in (jp, jp + 1):
                    jo = (j & 1) * 256
                    # scoresT: cols 0:256 = t0 x (s0|s1); cols 256:384 = t1 x s1
                    ps = psum.tile([128, 384], F32, tag="sm", bufs=1)
                    for ft in range(KT):
                        nc.tensor.matmul(
                            ps[:, 0:256],
                            kT_all[:, ft, j * 256: j * 256 + 128],
                            qT2[:, p2 + ft, jo: jo + 256],
                            start=(ft == 0), stop=False)
                        nc.tensor.matmul(
                            ps[:, 256:384],
                            kT_all[:, ft, j * 256 + 128: j * 256 + 256],
                            qT2[:, p2 + ft, jo + 128: jo + 256],
                            start=False, stop=(ft == KT - 1))
                    sm = sm_all[:, j * 384:(j + 1) * 384]
                    nc.vector.tensor_mul(sm[:, 0:128], ps[:, 0:128], tril)
                    nc.scalar.copy(out=sm[:, 128:256], in_=ps[:, 128:256])
                    nc.vector.tensor_mul(sm[:, 256:384], ps[:, 256:384], tril)

                    # den (local part): column sums of masked scores.
                    # NOTE: start=True clears the whole PSUM bank, so only the
                    # very first matmul into pden may carry it.
                    dc = j * CT
                    nc.tensor.matmul(pden[:, dc:dc + 1], sm[:, 0:128], ones_col_bf,
                                     start=(j == 0), stop=False)
                    nc.tensor.matmul(pden[:, dc + 1:dc + 2], sm[:, 128:256], ones_col_bf,
                                     start=False, stop=False)
                    nc.tensor.matmul(pden[:, dc + 1:dc + 2], sm[:, 256:384], ones_col_bf,
                                     start=False, stop=False)

                    # pair 0: also start local num (R-independent) to fill the
                    # exchange window; groups stay open into pass 2b.
                    if j < 2:
                        pn0 = psum.tile([128, 512], F32, tag="one", bufs=4)
                        pn1 = psum.tile([128, 512], F32, tag="one", bufs=4)
                        v0 = v_all[:, (j * CT + 0) * 512:(j * CT + 0) * 512 + 512]
                        v1 = v_all[:, (j * CT + 1) * 512:(j * CT + 1) * 512 + 512]
                        nc.tensor.matmul(pn0[:, :], sm[:, 0:128], v0,
                                         start=True, stop=False)
                        nc.tensor.matmul(pn1[:, :], sm[:, 128:256], v0,
                                         start=True, stop=False)
                        nc.tensor.matmul(pn1[:, :], sm[:, 256:384], v1,
                                         start=False, stop=False)
                        pn_pre[j] = (pn0, pn1)

            # ---------- pass 2b (needs R): num/den finalize + o-proj + LN2 ----------
            mv2 = small.tile([128, 2 * NT8], F32, tag="mv2")
            # exclusive-prefix states for all chunks (gated only on mR)
            Peffs = {0: mR}
            for j in range(1, NCH):
                Peff = work.tile([128, KT * SKW], BF16, tag="Peff", bufs=2)
                nc.vector.scalar_tensor_tensor(
                    out=Peff, in0=P_all[:, j * KT * SKW:(j + 1) * KT * SKW],
                    scalar=1.0, in1=mR, op0=ALU.mult, op1=ALU.add)
                Peffs[j] = Peff
            for jp in range(0, NCH, 2):
                p2 = (jp // 2) * KT
                for j in (jp, jp + 1):
                    jo = (j & 1) * 256
                    Peff = Peffs[j]

                    sm = sm_all[:, j * 384:(j + 1) * 384]
                    # num, seq-major: [s, e'] = masked-scores @ v + q @ KV.
                    # Order: local sm@v first (R-independent; pair 0's were
                    # already issued pre-wait), then the tiny den-finalize
                    # matmuls (so den/reciprocal overlaps the prefix
                    # matmuls), then q@KV prefix accumulation.
                    if j in pn_pre:
                        pn0, pn1 = pn_pre[j]
                    else:
                        pn0 = psum.tile([128, 512], F32, tag="one", bufs=4)
                        pn1 = psum.tile([128, 512], F32, tag="one", bufs=4)
                        v0 = v_all[:, (j * CT + 0) * 512:(j * CT + 0) * 512 + 512]
                        v1 = v_all[:, (j * CT + 1) * 512:(j * CT + 1) * 512 + 512]
                        nc.tensor.matmul(pn0[:, :], sm[:, 0:128], v0,
                                         start=True, stop=False)
                        nc.tensor.matmul(pn1[:, :], sm[:, 128:256], v0,
                                         start=True, stop=False)
                        nc.tensor.matmul(pn1[:, :], sm[:, 256:384], v1,
                                         start=False, stop=False)
                    dc = j * CT
                    for st in range(CT):
                        for kt in range(KT):
                            nc.tensor.matmul(
                                pden[:, dc + st:dc + st + 1],
                                qT2[:, p2 + kt, jo + st * 128: jo + st * 128 + 128],
                                Peff[:, kt * SKW + E: kt * SKW + SKW],
                                start=False, stop=(st == CT - 1 and kt == KT - 1))
                    den = small.tile([128, CT], F32, tag="den")
                    nc.scalar.activation(out=den, in_=pden[:, dc:dc + CT],
                                         func=AF.Identity,
                                         bias=eps_den, scale=1.0)
                    rden = small.tile([128, CT], F32, tag="rden")
                    nc.vector.reciprocal(out=rden, in_=den)
                    for kt in range(KT):
                        nc.tensor.matmul(pn0[:, :],
                                         qT2[:, p2 + kt, jo: jo + 128],
                                         Peff[:, kt * SKW: kt * SKW + E],
                                         start=False, stop=(kt == KT - 1))
                        nc.tensor.matmul(pn1[:, :],
                                         qT2[:, p2 + kt, jo + 128: jo + 256],
                                         Peff[:, kt * SKW: kt * SKW + E],
                                         start=False, stop=(kt == KT - 1))

                    # attn = num/den (seq-major, natural per-partition scale),
                    # then DMA-transpose to feature-major for the o-projection
                    attn = work.tile([128, CT * 512], BF16, tag="numT")
                    nc.scalar.activation(out=attn[:, 0:512], in_=pn0[:, :],
                                         func=AF.Copy, scale=rden[:, 0:1])
                    nc.scalar.activation(out=attn[:, 512:1024], in_=pn1[:, :],
                                         func=AF.Copy, scale=rden[:, 1:2])
                    attnT = work.tile([128, KT, 256], BF16, tag="attnT")
                    for st in range(CT):
                        nc.sync.dma_start_transpose(
                            out=attnT[:, :, st * 128:(st + 1) * 128],
                            in_=attn[:, st * 512:(st + 1) * 512])

                    # o-projection + residual
                    for st in range(CT):
                        po = psum.tile([128, 512], F32, tag="one", bufs=4)
                        for mt in range(KT):
                            nc.tensor.matmul(po[:, :],
                                             attnT[:, mt, st * 128:(st + 1) * 128],
                                             wo[:, (0 * KT + mt) * 512:(0 * KT + mt) * 512 + 512],
                                             start=(mt == 0),
                                             stop=(not with_bias and mt == KT - 1))
                        if with_bias:
                            nc.tensor.matmul(po[:, :], ones_row[:, 0:128],
                                             brow_w(i, 1), start=False, stop=True)
                        ls = lat[:, (j * CT + st) * 512:(j * CT + st) * 512 + 512]
                        nc.vector.scalar_tensor_tensor(out=ls, in0=po[:, :],
                                                       scalar=1.0,
                                                       in1=ls, op0=ALU.mult, op1=ALU.add)

                    # LN2 stats here (post-residual); scales batched per pair
                    for ct in range(CT):
                        ln_stats(mv2, j * CT + ct, j * CT + ct)

                # LN2 normalize + transpose sweep for this pair (DVE + DMA)
                negm2, rstd2 = ln_scales(mv2[:, jp * CT * 2: jp * CT * 2 + 8],
                                         4, "l2")
                for c4 in range(4):
                    ct8 = jp * CT + c4
                    y2 = work.tile([128, 512], BF16, tag="y2", bufs=2)
                    ln_norm(y2, ct8, negm2, rstd2, c4)
                    nc.sync.dma_start_transpose(
                        out=h2T_all[:, :, ct8 * 128:(ct8 + 1) * 128],
                        in_=y2)

            # ---------- pass 3: FFN (+ next layer's LN1, pipelined) ----------
            mv1n = small.tile([128, 2 * NT8], F32, tag="mv1")
            for jp in range(0, NCH, 2):
                h1T = work.tile([128, KT, 512], BF16, tag="h1T", bufs=1)
                for fh in range(2):
                    ph1 = psum.tile([128, 1024], F32, tag="big2", bufs=1)
                    for fi in range(2):
                        ft = fh * 2 + fi
                        for kt in range(KT):
                            nc.tensor.matmul(
                                ph1[:, fi * 512:(fi + 1) * 512],
                                wo[:, (1 * KT + kt) * 512 + ft * 128:(1 * KT + kt) * 512 + ft * 128 + 128],
                                h2T_all[:, kt, jp * 256: jp * 256 + 512],
                                start=(kt == 0), stop=(kt == KT - 1))
                    for fi in range(2):
                        ft = fh * 2 + fi
                        nc.scalar.activation(out=h1T[:, ft, :],
                                             in_=ph1[:, fi * 512:(fi + 1) * 512],
                                             func=AF.Gelu, bias=bcol_w(i, 2, ft),
                                             scale=1.0)

                for j in (jp, jp + 1):
                    jo = (j & 1) * 256
                    for st in range(CT):
                        pf = psum.tile([128, 512], F32, tag="one", bufs=4)
                        for mt in range(KT):
                            nc.tensor.matmul(
                                pf[:, :],
                                h1T[:, mt, jo + st * 128: jo + st * 128 + 128],
                                wo[:, (2 * KT + mt) * 512:(2 * KT + mt) * 512 + 512],
                                start=(mt == 0),
                                stop=(not with_bias and mt == KT - 1))
                        if with_bias:
                            nc.tensor.matmul(pf[:, :], ones_row[:, 0:128],
                                             brow_w(i, 2), start=False, stop=True)
                        ls = lat[:, (j * CT + st) * 512:(j * CT + st) * 512 + 512]
                        nc.vector.scalar_tensor_tensor(out=ls, in0=pf[:, :], scalar=1.0,
                                                       in1=ls, op0=ALU.mult, op1=ALU.add)
                        if i_rep < L * repeat - 1:
                            ln_stats(mv1n, j * CT + st, j * CT + st)
                if i_rep < L * repeat - 1:
                    ln1_sweep(mv1n, jp)
                else:
                    # last layer: fold the output projection into pass 3,
                    # per pair (tokens jp*256 .. jp*256+512)
                    latT = work.tile([128, KT, 512], BF16, tag="latT", bufs=2)
                    for c4 in range(4):
                        ct8 = jp * CT + c4
                        latb = work.tile([128, 512], BF16, tag="y2", bufs=2)
                        nc.vector.tensor_copy(out=latb,
                                              in_=lat[:, ct8 * 512:(ct8 + 1) * 512])
                        nc.sync.dma_start_transpose(
                            out=latT[:, :, c4 * 128:(c4 + 1) * 128], in_=latb)
                    pout = psum.tile([COUT, 512], F32, tag="one", bufs=4)
                    for kt in range(KT):
                        nc.tensor.matmul(pout[:, :],
                                         outWT[:, kt * COUT:(kt + 1) * COUT],
                                         latT[:, kt, :],
                                         start=(kt == 0),
                                         stop=(not with_bias and kt == KT - 1))
                    if with_bias:
                        nc.tensor.matmul(pout[:, :], outb_row, ones_row,
                                         start=False, stop=True)
                    out_sb = work.tile([COUT, 512], F32, tag="outsb", bufs=2)
                    nc.scalar.copy(out=out_sb, in_=pout[:, :])
                    dma(out=out_d[:, jp * 256: jp * 256 + 512], in_=out_sb)


# ---------------- host side ----------------

_CACHE = threading.local()


def _get_program(with_bias=False):
    key = f"nc_{with_bias}"
    if not hasattr(_CACHE, key):
        setattr(_CACHE, key, build_program(with_bias=with_bias))
    return getattr(_CACHE, key)


def _needs_bias(inputs):
    f32 = np.float32
    ln1_b = np.asarray(inputs["ln1_b"], f32)
    ln2_b = np.asarray(inputs["ln2_b"], f32)
    vals = [np.asarray(inputs[k], f32) for k in
            ("in_b", "cc_b", "out_b", "bo", "c2_b")]
    bv_eff = np.asarray(inputs["bv"], f32) + np.einsum(
        "loe,le->lo", np.asarray(inputs["Wv"], f32), ln1_b)
    vals.append(bv_eff)
    return any(np.abs(v).max() > 0 for v in vals)


def _prep_shared(inputs):
    f32 = np.float32
    inW = np.asarray(inputs["in_W"], f32)      # [E, CIN]
    in_b = np.asarray(inputs["in_b"], f32)
    ccW = np.asarray(inputs["cc_W"], f32)      # [E, E, 3]
    cc_b = np.asarray(inputs["cc_b"], f32)
    outW = np.asarray(inputs["out_W"], f32)    # [COUT, E]
    out_b = np.asarray(inputs["out_b"], f32)

    ccWT = np.zeros((128, 3 * KT * 512), f32)
    for d in range(3):
        WT = ccW[:, :, d].T  # [e_in, e_out]
        for kt in range(KT):
            ccWT[:, (d * KT + kt) * 512:(d * KT + kt) * 512 + 512] = \
                WT[kt * 128:(kt + 1) * 128, :]

    ln1_g = np.asarray(inputs["ln1_g"], f32); ln1_b = np.asarray(inputs["ln1_b"], f32)
    ln2_g = np.asarray(inputs["ln2_g"], f32); ln2_b = np.asarray(inputs["ln2_b"], f32)

    wpack = np.zeros((128, L * 6 * KT * 512), f32)
    brow = np.zeros((1, BROW_N), f32)
    bcol = np.zeros((128, L * 3 * KT), f32)
    for i in range(L):
        biases = {}
        for w, (Wn, bn, g, bb) in enumerate((
                ("Wq", "bq", ln1_g[i], ln1_b[i]),
                ("Wk", "bk", ln1_g[i], ln1_b[i]),
                ("Wv", "bv", ln1_g[i], ln1_b[i]),
                ("Wo", "bo", None, None),
                ("c1_W", "c1_b", ln2_g[i], ln2_b[i]),
                ("c2_W", "c2_b", None, None))):
            W = np.asarray(inputs[Wn], f32)[i]          # [E_out, E_in]
            bias = np.asarray(inputs[bn], f32)[i].copy()
            if g is not None:
                WT = (W * g[None, :]).T                  # fold LN gain
                bias = bias + W @ bb                     # fold LN bias
            else:
                WT = W.T
            for kt in range(KT):
                wpack[:, (i * 6 + w) * KT * 512 + kt * 512:
                      (i * 6 + w) * KT * 512 + kt * 512 + 512] = \
                    WT[kt * 128:(kt + 1) * 128, :]
            biases[w] = bias
        # rows: bv, bo, c2b
        brow[0, (i * 3 + 0) * 512:(i * 3 + 0) * 512 + 512] = biases[2]
        brow[0, (i * 3 + 1) * 512:(i * 3 + 1) * 512 + 512] = biases[3]
        brow[0, (i * 3 + 2) * 512:(i * 3 + 2) * 512 + 512] = biases[5]
        # cols: bq, bk, c1b
        for which, w in ((0, 0), (1, 1), (2, 4)):
            for kt in range(KT):
                bcol[:, (i * 3 + which) * KT + kt] = biases[w][kt * 128:(kt + 1) * 128]

    inWT = inW.T  # [CIN, E]
    outWT = np.zeros((128, KT * COUT), f32)
    for kt in range(KT):
        outWT[:, kt * COUT:(kt + 1) * COUT] = outW.T[kt * 128:(kt + 1) * 128, :]

    brow[0, L * 3 * 512: L * 3 * 512 + 512] = in_b
    brow[0, L * 3 * 512 + 512: L * 3 * 512 + 1024] = cc_b
    brow[0, L * 3 * 512 + 1024: L * 3 * 512 + 1024 + COUT] = out_b

    tril = np.tril(np.ones((128, 128), f32)).T  # keep t<=s in [t,s] layout

    return {
        "inWT": inWT.astype(BF),
        "ccWT": ccWT.astype(BF),
        "wpack": wpack.astype(BF),
        "outWT": outWT.astype(BF),
        "brow": brow.astype(BF),
        "bcol": bcol,
        "tril": tril.astype(BF),
        "ones_row": np.ones((1, 512), f32).astype(BF),
        "ones_col_bf": np.ones((128, 1), f32).astype(BF),
    }


def _prep_core_inputs(shared, inputs, b, h):
    f32 = np.float32
    x = np.asarray(inputs["x"], f32)
    s0 = h * SH
    x_sl = np.zeros((CIN, SH + 2), f32)
    lo = max(0, s0 - 2)
    x_sl[:, 2 - (s0 - lo):] = x[b, :, lo:s0 + SH]
    halo = np.ones((1, SH + 2), f32)
    if h == 0:
        halo[0, :2] = 0.0
    mcol = np.zeros((128, 2), f32)
    mcol[:, 0] = float(h)
    mcol[:, 1] = 1.0 - float(h)
    m = dict(shared)
    m["x_sl"] = x_sl.astype(BF)
    m["halo"] = halo.astype(BF)
    m["mcol"] = mcol
    return m


def _run(inputs, **kw):
    nc = _get_program(with_bias=_needs_bias(inputs))
    shared = _prep_shared(inputs)
    in_maps = []
    for core in range(NCORES):
        b, h = core // 2, core % 2
        in_maps.append(_prep_core_inputs(shared, inputs, b, h))
    return run_bass_kernel_spmd(nc, in_maps, core_ids=list(range(NCORES)), **kw)


def kernel(**inputs):
    res = _run(inputs)
    out = np.zeros((B, COUT, S), np.float32)
    for core in range(NCORES):
        b, h = core // 2, core % 2
        out[b, :, h * SH:(h + 1) * SH] = res.results[core]["out"]
    return out


def bench(inputs, trace_cores=(0, 1), tmpdir=None):
    """Run with NTFF tracing; returns BassKernelResults with exec_time_ns."""
    return _run(inputs, trace=True, trace_cores=list(trace_cores), tmpdir=tmpdir)

